# revision 10
# baseline (speedup 1.0000x reference)
"""CHQS deconvolution kernel for Trainium2 (8 NeuronCores).

Reference computation: 5 outer iterations of
  z = softshrink(G x, lam_i); then 2x { r0 = y - Kx; r1 = z - Gx;
  x += D_i * pad([r0, r1], 2) }; x = clip(x, 0, 1)
with K a 31x31 blur (replicate pad 15), G 2-channel finite-diff (pad 2),
D_i [3,5,5] (pad 2).

Implementation identity (replicate-pad is linear: pad(a-b) = pad(a)-pad(b)):
  x_new = x + w - D0*pad(Kx) - CC*x
  w  = D0*pad(y) + D1*pad(z1) + D2*pad(z2)    (per outer iteration)
  CC = D1*G1 + D2*G2                          (composed 9x9 conv)
All convs run on the tensor engine as per-kernel-row Toeplitz matmuls in
float32r (full-rate, ~1e-4 accurate). Image layout transposed:
[cols -> partitions, rows -> free]; 98-col slabs; conv inputs are
DMA-built 128/102-col windows. Horizontal replicate-pad = replicated
edge-col tiles; vertical replicate-pad = maintained pad rows (masked so
only the true image top/bottom cores blend).

Sharding: 8 cores x (H/8) output rows, one launch per outer iteration
(one compiled program; d_i-dependent Toeplitz tables are inputs).
"""

import math
import numpy as np

import concourse.bacc as bacc
import concourse.mybir as mybir
import concourse.tile as tile
from concourse.bass_utils import run_bass_kernel_spmd

F32 = mybir.dt.float32
F32R = mybir.dt.float32r

N_ITER = 5
N_IN = 2
LAMBD = 0.005
BETA = (np.array([0.0, 1.0, 4.0, 16.0, 64.0, 256.0, 1024.0, 4096.0,
                  16384.0, 65536.0]) * 0.001 / 10.0 * 81.0)

KS, HKS = 31, 15
DS, HDS = 5, 2
S = HKS + HDS  # 17 halo rows consumed per inner step


def _toeplitz(rows, win_w, out_w, pad, R):
    A, T = rows.shape
    tabs = np.zeros((A, win_w, out_w), dtype=np.float32)
    for j in range(out_w):
        for t in range(T):
            c = j + pad + t - R
            if 0 <= c < win_w:
                tabs[:, c, j] = rows[:, t]
    return tabs


def _flat(tabs):
    A, P, O = tabs.shape
    return np.ascontiguousarray(tabs.transpose(1, 0, 2)).reshape(P, A * O)


def _g_offsets(g):
    nz = [a for a in range(DS) if np.any(g[a] != 0)]
    return [a - HDS for a in (nz or [HDS])]


def _cc_offsets(weight):
    offs = {0}
    for chn in (0, 1):
        for a in range(DS):
            if np.any(weight[chn, 0, a] != 0):
                for p in range(DS):
                    offs.add((p - HDS) + (a - HDS))
    return sorted(offs)


def make_tables(k2d, d_i, weight, offsG1, offsG2):
    tabK = _flat(_toeplitz(k2d, 128, 98, HKS, HKS))
    g1, g2 = weight[0, 0], weight[1, 0]
    tG1 = _flat(_toeplitz(np.stack([g1[v + HDS] for v in offsG1]),
                          128, 98, HKS, HDS))
    tG2 = _flat(_toeplitz(np.stack([g2[v + HDS] for v in offsG2]),
                          128, 98, HKS, HDS))
    tIx = _flat(_toeplitz(np.ones((1, 1), np.float32), 128, 98, HKS, 0))
    return dict(
        tabK=tabK, tabIx=tIx, tabG1=tG1, tabG2=tG2,
        tabD0y=_flat(_toeplitz(d_i[0], 102, 98, HDS, HDS)),
        tabD1=_flat(_toeplitz(d_i[1], 102, 98, HDS, HDS)),
        tabD2=_flat(_toeplitz(d_i[2], 102, 98, HDS, HDS)),
        tabD0=_flat(_toeplitz(-d_i[0], 102, 98, HDS, HDS)),
        tabD1n=_flat(_toeplitz(-d_i[1], 102, 98, HDS, HDS)),
        tabD2n=_flat(_toeplitz(-d_i[2], 102, 98, HDS, HDS)),
        idw=np.eye(98, dtype=np.float32))


class Builder:
    def __init__(self, W, OWN, n_ch, n_cores, offsG1, offsG2):
        self.W, self.OWN, self.n_ch, self.n_cores = W, OWN, n_ch, n_cores
        self.NB = math.ceil(W / 98)
        self.WPAD = self.NB * 98
        self.BAND = OWN + 4 * S
        self.L1 = OWN + 2 * S
        self.LW = self.L1
        self.LZ = self.LW + 2 * HDS
        self.offsG1, self.offsG2 = offsG1, offsG2
        self.fake = self.WPAD - W
        self.ERW = HKS + self.fake
        self.ERS = HDS + self.fake

    def build(self):
        W, NB, BAND, n_ch = self.W, self.NB, self.BAND, self.n_ch
        LW, LZ = self.LW, self.LZ
        offsG1, offsG2 = self.offsG1, self.offsG2
        nvG1, nvG2 = len(offsG1), len(offsG2)
        ERW, ERS = self.ERW, self.ERS
        zr0 = S - HDS
        lastb, lastp = divmod(W - 1, 98)

        nc = bacc.Bacc("TRN2", target_bir_lowering=False, debug=False,
                       num_devices=self.n_cores)
        din = lambda n, s, dt=F32R: nc.dram_tensor(
            n, s, dt, kind="ExternalInput").ap()
        x_d = din("x", (n_ch, self.WPAD, BAND))
        y_d = din("y", (n_ch, self.WPAD, BAND))
        tabK_d = din("tabK", (128, KS * 98))
        tabIx_d = din("tabIx", (128, 98))
        tabG1_d = din("tabG1", (128, nvG1 * 98))
        tabG2_d = din("tabG2", (128, nvG2 * 98))
        tabD0y_d = din("tabD0y", (102, DS * 98))
        tabD1_d = din("tabD1", (102, DS * 98))
        tabD2_d = din("tabD2", (102, DS * 98))
        tabD0_d = din("tabD0", (102, DS * 98))
        tabD1n_d = din("tabD1n", (102, DS * 98))
        tabD2n_d = din("tabD2n", (102, DS * 98))
        idw_d = din("idw", (98, 98))
        lam_d = din("lam", (98, 1), F32)
        mtop_d = din("mtop", (98, 1), F32)
        mbot_d = din("mbot", (98, 1), F32)
        out_d = nc.dram_tensor("o", (n_ch, self.WPAD, self.OWN), F32,
                               kind="ExternalOutput").ap()

        def pieces(c0, ww):
            res, c = [], c0
            while c < c0 + ww:
                if c < 0:
                    n = min(-c, c0 + ww - c)
                    res.append((c - c0, "L", 0, n))
                elif c >= W:
                    n = c0 + ww - c
                    res.append((c - c0, "R", 0, n))
                else:
                    b, p = divmod(c, 98)
                    n = min(98 - p, c0 + ww - c, W - c)
                    res.append((c - c0, "S", c, n))
                c += n
            return res

        xplan = [pieces(98 * B - HKS, 128) for B in range(NB)]
        splan = [pieces(98 * B - HDS, 102) for B in range(NB)]

        with tile.TileContext(nc) as tc:
            with tc.tile_pool(name="tabs", bufs=1) as tabp, \
                 tc.tile_pool(name="mast", bufs=1) as mast, \
                 tc.tile_pool(name="xw", bufs=NB + 2) as xwp, \
                 tc.tile_pool(name="sw", bufs=4) as swp, \
                 tc.tile_pool(name="zsl", bufs=4) as zslp, \
                 tc.tile_pool(name="edg", bufs=2) as edgp, \
                 tc.tile_pool(name="ps", bufs=3, space="PSUM") as pp, \
                 tc.tile_pool(name="ps2", bufs=3, space="PSUM") as pp2:

                def load_tab(d, p, w_, tag):
                    t = tabp.tile([p, w_], F32R, tag=tag)
                    nc.sync.dma_start(out=t[:, :], in_=d[:, :])
                    return t

                tK = load_tab(tabK_d, 128, KS * 98, "tK")
                tIx = load_tab(tabIx_d, 128, 98, "tIx")
                tG1 = load_tab(tabG1_d, 128, nvG1 * 98, "tG1")
                tG2 = load_tab(tabG2_d, 128, nvG2 * 98, "tG2")
                tD0y = load_tab(tabD0y_d, 102, DS * 98, "tD0y")
                tD1 = load_tab(tabD1_d, 102, DS * 98, "tD1")
                tD2 = load_tab(tabD2_d, 102, DS * 98, "tD2")
                tD0 = load_tab(tabD0_d, 102, DS * 98, "tD0")
                tD1n = load_tab(tabD1n_d, 102, DS * 98, "tD1n")
                tD2n = load_tab(tabD2n_d, 102, DS * 98, "tD2n")
                tIw = load_tab(idw_d, 98, 98, "tIw")
                lam = tabp.tile([98, 1], F32, tag="lam")
                nc.sync.dma_start(out=lam[:, :], in_=lam_d[:, :])
                mtop = tabp.tile([98, 1], F32, tag="mtop")
                nc.sync.dma_start(out=mtop[:, :], in_=mtop_d[:, :])
                mbot = tabp.tile([98, 1], F32, tag="mbot")
                nc.sync.dma_start(out=mbot[:, :], in_=mbot_d[:, :])

                def doubling(t, width):
                    k = 1
                    while k < width:
                        n = min(k, width - k)
                        nc.sync.dma_start(out=t[k:k + n, :], in_=t[0:n, :])
                        k += n

                def edge_from(ap_onecol, width, rows, tag):
                    e = edgp.tile([width, rows], F32R, tag=tag)
                    nc.sync.dma_start(out=e[0:1, :], in_=ap_onecol)
                    doubling(e, width)
                    return e

                def blend(out_ap, src_1col, mask, tmp_pool_tag, shape):
                    tmp = zslp.tile(list(shape), F32R, tag=tmp_pool_tag)
                    t_ap = tmp[tuple(slice(0, d_) for d_ in shape)]
                    nc.vector.tensor_sub(t_ap, src_1col.broadcast_to(shape),
                                         out_ap)
                    nc.vector.scalar_tensor_tensor(
                        out_ap, t_ap, mask[:, :1], out_ap,
                        mybir.AluOpType.mult, mybir.AluOpType.add)

                def softshrink(dst, src_ap, tmp):
                    nc.vector.tensor_scalar(dst[:, :], src_ap, lam[:, :1],
                                            0.0, mybir.AluOpType.subtract,
                                            mybir.AluOpType.max)
                    nc.vector.tensor_scalar(tmp[:, :], src_ap, lam[:, :1],
                                            0.0, mybir.AluOpType.add,
                                            mybir.AluOpType.min)
                    nc.vector.tensor_add(dst[:, :], dst[:, :], tmp[:, :])

                for ch in range(n_ch):
                    xs = mast.tile([98, NB, BAND], F32R, tag="xs")
                    for b in range(NB):
                        nc.sync.dma_start(
                            out=xs[:, b, :],
                            in_=x_d[ch, 98 * b:98 * b + 98, :])
                    ws = mast.tile([98, NB, LW], F32R, tag="ws")

                    # ---- x0 windows (serve z/w phase AND step 1) ----
                    eLx = edge_from(xs[0:1, 0, :], HKS, BAND, "xeL")
                    eRx = edge_from(xs[lastp:lastp + 1, lastb, :], ERW, BAND,
                                    "xeR")

                    def xwin(B, base, nrows, wb):
                        win = xwp.tile([128, nrows], F32R, tag="xw")
                        for dst, kind, off, n in xplan[B]:
                            if kind == "S":
                                b, p = divmod(off, 98)
                                nc.sync.dma_start(
                                    out=win[dst:dst + n, :],
                                    in_=xs[p:p + n, b,
                                           base:base + nrows])
                            elif kind == "L":
                                nc.sync.dma_start(
                                    out=win[dst:dst + n, :],
                                    in_=eLx[0:n, base:base + nrows]
                                     if wb is None else
                                    wb[0][0:n, base:base + nrows])
                            else:
                                nc.sync.dma_start(
                                    out=win[dst:dst + n, :],
                                    in_=eRx[0:n, base:base + nrows]
                                     if wb is None else
                                    wb[1][0:n, base:base + nrows])
                        return win

                    xw0 = {B: xwin(B, 0, BAND, None) for B in range(NB)}

                    def swin(plan_B, slab_ap, eL, eR, rows, tag):
                        win = swp.tile([102, rows], F32R, tag=tag)
                        for dst, kind, off, n in plan_B:
                            if kind == "S":
                                nc.sync.dma_start(out=win[dst:dst + n, :],
                                                  in_=slab_ap(off, n))
                            elif kind == "L":
                                nc.sync.dma_start(out=win[dst:dst + n, :],
                                                  in_=eL[0:n, :rows])
                            else:
                                nc.sync.dma_start(out=win[dst:dst + n, :],
                                                  in_=eR[0:n, :rows])
                        return win

                    # ======== z/w phase ========
                    z1s, z2s = {}, {}

                    def make_z(B):
                        psz = pp.tile([98, LZ], F32, tag="acc1")
                        for i, v in enumerate(offsG1):
                            nc.tensor.matmul(psz[:, :],
                                             tG1[:, 98 * i:98 * i + 98],
                                             xw0[B][:, zr0 + v:zr0 + v + LZ],
                                             start=(i == 0),
                                             stop=(i == nvG1 - 1))
                        z1 = zslp.tile([98, LZ], F32R, tag="z1")
                        zt = zslp.tile([98, LZ], F32R, tag="zt")
                        softshrink(z1, psz[:, :], zt)
                        psz2 = pp.tile([98, LZ], F32, tag="acc1")
                        for i, v in enumerate(offsG2):
                            nc.tensor.matmul(psz2[:, :],
                                             tG2[:, 98 * i:98 * i + 98],
                                             xw0[B][:, zr0 + v:zr0 + v + LZ],
                                             start=(i == 0),
                                             stop=(i == nvG2 - 1))
                        z2 = zslp.tile([98, LZ], F32R, tag="z2")
                        softshrink(z2, psz2[:, :], zt)
                        for zz in (z1, z2):
                            td = S
                            blend(zz[:, td:td + 2], zz[:, td + 2:td + 3],
                                  mtop, "btmp", (98, 2))
                            bd = BAND - 2 * S - (S - 2)
                            blend(zz[:, bd:bd + 2], zz[:, bd - 1:bd],
                                  mbot, "btmp", (98, 2))
                        z1s[B], z2s[B] = z1, z2

                    yeL = edge_from(y_d[ch, 0:1, zr0:zr0 + LZ], HDS, LZ,
                                    "yeL")
                    yeR = edge_from(y_d[ch, W - 1:W, zr0:zr0 + LZ], ERS, LZ,
                                    "yeR")
                    make_z(0)
                    ez1L = ez2L = ez1R = ez2R = None
                    for B in range(NB):
                        if B + 1 < NB:
                            make_z(B + 1)
                        if ez1L is None:
                            ez1L = edge_from(z1s[0][0:1, :], HDS, LZ, "z1L")
                            ez2L = edge_from(z2s[0][0:1, :], HDS, LZ, "z2L")
                        if B == NB - 1 and ez1R is None:
                            ez1R = edge_from(z1s[lastb][lastp:lastp + 1, :],
                                             ERS, LZ, "z1R")
                            ez2R = edge_from(z2s[lastb][lastp:lastp + 1, :],
                                             ERS, LZ, "z2R")
                        psw = pp2.tile([98, LW], F32, tag="acc2")
                        yw = swin(splan[B],
                                  lambda off, n: y_d[
                                      ch, off:off + n,
                                      zr0:zr0 + LZ],
                                  yeL, yeR, LZ, "yw")
                        for a in range(DS):
                            nc.tensor.matmul(psw[:, :],
                                             tD0y[:, 98 * a:98 * a + 98],
                                             yw[:, a:a + LW],
                                             start=(a == 0), stop=False)
                        z1w = swin(splan[B],
                                   lambda off, n: z1s[off // 98][
                                       off % 98:off % 98 + n, :],
                                   ez1L, ez1R, LZ, "z1w")
                        for a in range(DS):
                            nc.tensor.matmul(psw[:, :],
                                             tD1[:, 98 * a:98 * a + 98],
                                             z1w[:, a:a + LW],
                                             start=False, stop=False)
                        z2w = swin(splan[B],
                                   lambda off, n: z2s[off // 98][
                                       off % 98:off % 98 + n, :],
                                   ez2L, ez2R, LZ, "z2w")
                        for a in range(DS):
                            nc.tensor.matmul(psw[:, :],
                                             tD2[:, 98 * a:98 * a + 98],
                                             z2w[:, a:a + LW],
                                             start=False, stop=(a == DS - 1))
                        nc.vector.tensor_copy(ws[:, B, :], psw[:, :])
                        z1s.pop(B - 1, None)
                        z2s.pop(B - 1, None)

                    # ======== inner steps ========
                    for step in range(N_IN):
                        r0 = S * (step + 1)
                        L = BAND - 2 * r0
                        if step == 0:
                            wbase, xws = 0, xw0
                        else:
                            wbase = r0 - S
                            eLx2 = edge_from(xs[0:1, 0, :], HKS, BAND, "xeL")
                            eRx2 = edge_from(xs[lastp:lastp + 1, lastb, :],
                                             ERW, BAND, "xeR")
                            xws = {B: xwin(B, wbase, L + 4 + 2 * HKS,
                                           (eLx2, eRx2)) for B in range(NB)}
                        kxs = {}
                        g1s, g2s = {}, {}
                        ekxL = ekxR = None
                        eg = {}

                        def make_g(B):
                            for (tG, nvG, offsG, dct, tag) in (
                                    (tG1, nvG1, offsG1, g1s, "g1"),
                                    (tG2, nvG2, offsG2, g2s, "g2")):
                                psg = pp.tile([98, L + 4], F32, tag="acc1")
                                for i, v in enumerate(offsG):
                                    o = r0 - 2 + v - wbase
                                    nc.tensor.matmul(
                                        psg[:, :], tG[:, 98 * i:98 * i + 98],
                                        xws[B][:, o:o + L + 4],
                                        start=(i == 0), stop=(i == nvG - 1))
                                g = zslp.tile([98, L + 4], F32R, tag=tag)
                                nc.vector.tensor_copy(g[:, :], psg[:, :])
                                td = 2 * S - r0
                                blend(g[:, td:td + 2], g[:, td + 2:td + 3],
                                      mtop, "btmp", (98, 2))
                                bd = BAND - 2 * S - (r0 - 2)
                                blend(g[:, bd:bd + 2], g[:, bd - 1:bd],
                                      mbot, "btmp", (98, 2))
                                dct[B] = g

                        def make_kx(B):
                            psk = pp.tile([98, L + 4], F32, tag="acc1")
                            for a in range(KS):
                                o = a + r0 - wbase - HKS - HDS
                                nc.tensor.matmul(psk[:, :],
                                                 tK[:, 98 * a:98 * a + 98],
                                                 xws[B][:, o:o + L + 4],
                                                 start=(a == 0),
                                                 stop=(a == KS - 1))
                            kx = zslp.tile([98, L + 4], F32R, tag="kx")
                            nc.vector.tensor_copy(kx[:, :], psk[:, :])
                            # vertical replicate blend: global rows -2,-1 <-
                            # row 0 (core 0); rows H, H+1 <- H-1 (last core)
                            td = 2 * S - r0
                            blend(kx[:, td:td + 2], kx[:, td + 2:td + 3],
                                  mtop, "btmp", (98, 2))
                            bd = BAND - 2 * S - (r0 - 2)
                            blend(kx[:, bd:bd + 2], kx[:, bd - 1:bd],
                                  mbot, "btmp", (98, 2))
                            kxs[B] = kx

                        make_kx(0)
                        make_g(0)
                        for B in range(NB):
                            if B + 1 < NB:
                                make_kx(B + 1)
                                make_g(B + 1)
                            if ekxL is None:
                                ekxL = edge_from(kxs[0][0:1, :], HDS, L + 4,
                                                 "kxL")
                                eg["g1L"] = edge_from(g1s[0][0:1, :], HDS,
                                                      L + 4, "g1L")
                                eg["g2L"] = edge_from(g2s[0][0:1, :], HDS,
                                                      L + 4, "g2L")
                            if B == NB - 1 and ekxR is None:
                                ekxR = edge_from(
                                    kxs[lastb][lastp:lastp + 1, :], ERS,
                                    L + 4, "kxR")
                                eg["g1R"] = edge_from(
                                    g1s[lastb][lastp:lastp + 1, :], ERS,
                                    L + 4, "g1R")
                                eg["g2R"] = edge_from(
                                    g2s[lastb][lastp:lastp + 1, :], ERS,
                                    L + 4, "g2R")
                            psx = pp2.tile([98, L], F32, tag="acc2")
                            nc.tensor.matmul(
                                psx[:, :], tIw[:, :],
                                ws[:, B, r0 - S:r0 - S + L],
                                start=True, stop=False)
                            nc.tensor.matmul(psx[:, :], tIx[:, :98],
                                             xws[B][:, r0 - wbase:
                                                    r0 - wbase + L],
                                             start=False, stop=False)
                            kw = swin(splan[B],
                                      lambda off, n: kxs[off // 98][
                                          off % 98:off % 98 + n, :],
                                      ekxL, ekxR, L + 4, "kw")
                            for a in range(DS):
                                nc.tensor.matmul(psx[:, :],
                                                 tD0[:, 98 * a:98 * a + 98],
                                                 kw[:, a:a + L],
                                                 start=False, stop=False)
                            g1w = swin(splan[B],
                                       lambda off, n: g1s[off // 98][
                                           off % 98:off % 98 + n, :],
                                       eg.get("g1L"), eg.get("g1R"), L + 4, "g1w")
                            for a in range(DS):
                                nc.tensor.matmul(psx[:, :],
                                                 tD1n[:, 98 * a:98 * a + 98],
                                                 g1w[:, a:a + L],
                                                 start=False, stop=False)
                            g2w = swin(splan[B],
                                       lambda off, n: g2s[off // 98][
                                           off % 98:off % 98 + n, :],
                                       eg.get("g2L"), eg.get("g2R"), L + 4, "g2w")
                            for a in range(DS):
                                nc.tensor.matmul(psx[:, :],
                                                 tD2n[:, 98 * a:98 * a + 98],
                                                 g2w[:, a:a + L],
                                                 start=False,
                                                 stop=(a == DS - 1))
                            if step == N_IN - 1:
                                nc.vector.tensor_scalar(
                                    xs[:, B, r0:r0 + L], psx[:, :], 0.0, 1.0,
                                    mybir.AluOpType.max, mybir.AluOpType.min)
                            else:
                                nc.vector.tensor_copy(xs[:, B, r0:r0 + L],
                                                      psx[:, :])
                            kxs.pop(B - 1, None)
                            g1s.pop(B - 1, None)
                            g2s.pop(B - 1, None)
                        if step < N_IN - 1:
                            blend(xs[:, :, S:2 * S],
                                  xs[:, :, 2 * S:2 * S + 1], mtop,
                                  "btmp3", (98, NB, S))
                            blend(xs[:, :, BAND - 2 * S:BAND - S],
                                  xs[:, :, BAND - 2 * S - 1:BAND - 2 * S],
                                  mbot, "btmp3", (98, NB, S))

                    for b in range(NB):
                        nc.sync.dma_start(
                            out=out_d[ch, 98 * b:98 * b + 98, :],
                            in_=xs[:, b, 2 * S:2 * S + self.OWN].bitcast(F32))

        nc.compile()
        return nc


LAST_EXEC_NS = None


def run_chqs(input_img, k, d, weight, n_cores=8, runner=None, trace=False):
    B0, C, H, W = input_img.shape
    OWN = H // n_cores
    k2d = np.asarray(k, np.float32)[0, 0]
    d = np.asarray(d, np.float32)
    weight = np.asarray(weight, np.float32)
    offsG1 = _g_offsets(weight[0, 0])
    offsG2 = _g_offsets(weight[1, 0])
    bld = Builder(W, OWN, C, n_cores, offsG1, offsG2)
    nc = bld.build()
    NB, WPAD = bld.NB, bld.WPAD

    img = np.asarray(input_img, np.float32)[0]

    def to_planes(a):
        t = np.transpose(a, (0, 2, 1))
        if WPAD > W:
            t = np.concatenate(
                [t, np.repeat(t[:, W - 1:W, :], WPAD - W, axis=1)], axis=1)
        return np.ascontiguousarray(t)

    y_pl = to_planes(img)

    def band_of(pl, c):
        idx = np.clip(np.arange(OWN * c - 2 * S, OWN * c + OWN + 2 * S),
                      0, H - 1)
        return np.ascontiguousarray(pl[:, :, idx])

    y_bands = [band_of(y_pl, c) for c in range(n_cores)]
    mt = [np.full((98, 1), 1.0 if c == 0 else 0.0, np.float32)
          for c in range(n_cores)]
    mb = [np.full((98, 1), 1.0 if c == n_cores - 1 else 0.0, np.float32)
          for c in range(n_cores)]

    x_pl = y_pl.copy()
    for it in range(N_ITER):
        lamv = LAMBD / max(1e-4, float(BETA[it]))
        tabs = make_tables(k2d, d[it], weight, offsG1, offsG2)
        in_maps = []
        for c in range(n_cores):
            m = dict(tabs)
            m["x"] = band_of(x_pl, c)
            m["y"] = y_bands[c]
            m["lam"] = np.full((98, 1), lamv, np.float32)
            m["mtop"] = mt[c]
            m["mbot"] = mb[c]
            in_maps.append(m)
        if runner is None:
            res = run_bass_kernel_spmd(nc, in_maps, list(range(n_cores)),
                                       trace=trace)
            outs = res.results
            if res.exec_time_ns:
                global LAST_EXEC_NS
                LAST_EXEC_NS = (LAST_EXEC_NS or 0) + res.exec_time_ns
        else:
            outs = runner(nc, in_maps)
        for c in range(n_cores):
            x_pl[:, :, OWN * c:OWN * c + OWN] = outs[c]["o"]
    return np.ascontiguousarray(
        np.transpose(x_pl[:, :W, :], (0, 2, 1)))[None].astype(np.float32)


def kernel(input, k, d, weight):
    return run_chqs(input, k, d, weight, n_cores=8)


# revision 13
# speedup vs baseline: 1.0018x; 1.0018x over previous
"""CHQS deconvolution kernel for Trainium2 (8 NeuronCores).

Reference computation: 5 outer iterations of
  z = softshrink(G x, lam_i); then 2x { r0 = y - Kx; r1 = z - Gx;
  x += D_i * pad([r0, r1], 2) }; x = clip(x, 0, 1)
with K a 31x31 blur (replicate pad 15), G 2-channel finite-diff (pad 2),
D_i [3,5,5] (pad 2).

Implementation identity (replicate-pad is linear: pad(a-b) = pad(a)-pad(b)):
  x_new = x + w - D0*pad(Kx) - CC*x
  w  = D0*pad(y) + D1*pad(z1) + D2*pad(z2)    (per outer iteration)
  CC = D1*G1 + D2*G2                          (composed 9x9 conv)
All convs run on the tensor engine as per-kernel-row Toeplitz matmuls in
float32r (full-rate, ~1e-4 accurate). Image layout transposed:
[cols -> partitions, rows -> free]; 98-col slabs; conv inputs are
DMA-built 128/102-col windows. Horizontal replicate-pad = replicated
edge-col tiles; vertical replicate-pad = maintained pad rows (masked so
only the true image top/bottom cores blend).

Sharding: 8 cores x (H/8) output rows, one launch per outer iteration
(one compiled program; d_i-dependent Toeplitz tables are inputs).
"""

import math
import numpy as np

import concourse.bacc as bacc
import concourse.mybir as mybir
import concourse.tile as tile
from concourse.bass_utils import run_bass_kernel_spmd

F32 = mybir.dt.float32
F32R = mybir.dt.float32r

N_ITER = 5
N_IN = 2
LAMBD = 0.005
BETA = (np.array([0.0, 1.0, 4.0, 16.0, 64.0, 256.0, 1024.0, 4096.0,
                  16384.0, 65536.0]) * 0.001 / 10.0 * 81.0)

KS, HKS = 31, 15
DS, HDS = 5, 2
S = HKS + HDS  # 17 halo rows consumed per inner step


def _toeplitz(rows, win_w, out_w, pad, R):
    A, T = rows.shape
    tabs = np.zeros((A, win_w, out_w), dtype=np.float32)
    for j in range(out_w):
        for t in range(T):
            c = j + pad + t - R
            if 0 <= c < win_w:
                tabs[:, c, j] = rows[:, t]
    return tabs


def _flat(tabs):
    A, P, O = tabs.shape
    return np.ascontiguousarray(tabs.transpose(1, 0, 2)).reshape(P, A * O)


def _g_offsets(g):
    nz = [a for a in range(DS) if np.any(g[a] != 0)]
    return [a - HDS for a in (nz or [HDS])]


def _cc_offsets(weight):
    offs = {0}
    for chn in (0, 1):
        for a in range(DS):
            if np.any(weight[chn, 0, a] != 0):
                for p in range(DS):
                    offs.add((p - HDS) + (a - HDS))
    return sorted(offs)


def make_tables(k2d, d_i, weight, offsG1, offsG2):
    tabK = _flat(_toeplitz(k2d, 128, 98, HKS, HKS))
    g1, g2 = weight[0, 0], weight[1, 0]
    tG1 = _flat(_toeplitz(np.stack([g1[v + HDS] for v in offsG1]),
                          128, 98, HKS, HDS))
    tG2 = _flat(_toeplitz(np.stack([g2[v + HDS] for v in offsG2]),
                          128, 98, HKS, HDS))
    tIx = _flat(_toeplitz(np.ones((1, 1), np.float32), 128, 98, HKS, 0))
    return dict(
        tabK=tabK, tabIx=tIx, tabG1=tG1, tabG2=tG2,
        tabD0y=_flat(_toeplitz(d_i[0], 102, 98, HDS, HDS)),
        tabD1=_flat(_toeplitz(d_i[1], 102, 98, HDS, HDS)),
        tabD2=_flat(_toeplitz(d_i[2], 102, 98, HDS, HDS)),
        tabD0=_flat(_toeplitz(-d_i[0], 102, 98, HDS, HDS)),
        tabD1n=_flat(_toeplitz(-d_i[1], 102, 98, HDS, HDS)),
        tabD2n=_flat(_toeplitz(-d_i[2], 102, 98, HDS, HDS)),
        idw=np.eye(98, dtype=np.float32))


class Builder:
    def __init__(self, W, OWN, n_ch, n_cores, offsG1, offsG2):
        self.W, self.OWN, self.n_ch, self.n_cores = W, OWN, n_ch, n_cores
        self.NB = math.ceil(W / 98)
        self.WPAD = self.NB * 98
        self.BAND = OWN + 4 * S
        self.L1 = OWN + 2 * S
        self.LW = self.L1
        self.LZ = self.LW + 2 * HDS
        self.offsG1, self.offsG2 = offsG1, offsG2
        self.fake = self.WPAD - W
        self.ERW = HKS + self.fake
        self.ERS = HDS + self.fake

    def build(self):
        W, NB, BAND, n_ch = self.W, self.NB, self.BAND, self.n_ch
        LW, LZ = self.LW, self.LZ
        offsG1, offsG2 = self.offsG1, self.offsG2
        nvG1, nvG2 = len(offsG1), len(offsG2)
        ERW, ERS = self.ERW, self.ERS
        zr0 = S - HDS
        lastb, lastp = divmod(W - 1, 98)

        nc = bacc.Bacc("TRN2", target_bir_lowering=False, debug=False,
                       num_devices=self.n_cores)
        din = lambda n, s, dt=F32R: nc.dram_tensor(
            n, s, dt, kind="ExternalInput").ap()
        x_d = din("x", (n_ch, self.WPAD, BAND))
        y_d = din("y", (n_ch, self.WPAD, BAND))
        tabK_d = din("tabK", (128, KS * 98))
        tabIx_d = din("tabIx", (128, 98))
        tabG1_d = din("tabG1", (128, nvG1 * 98))
        tabG2_d = din("tabG2", (128, nvG2 * 98))
        tabD0y_d = din("tabD0y", (102, DS * 98))
        tabD1_d = din("tabD1", (102, DS * 98))
        tabD2_d = din("tabD2", (102, DS * 98))
        tabD0_d = din("tabD0", (102, DS * 98))
        tabD1n_d = din("tabD1n", (102, DS * 98))
        tabD2n_d = din("tabD2n", (102, DS * 98))
        idw_d = din("idw", (98, 98))
        lam_d = din("lam", (98, 1), F32)
        mtop_d = din("mtop", (98, 1), F32)
        mbot_d = din("mbot", (98, 1), F32)
        mti_d = din("mti", (98, 2), mybir.dt.int32)
        mbi_d = din("mbi", (98, 2), mybir.dt.int32)
        out_d = nc.dram_tensor("o", (n_ch, self.WPAD, self.OWN), F32,
                               kind="ExternalOutput").ap()

        def pieces(c0, ww):
            res, c = [], c0
            while c < c0 + ww:
                if c < 0:
                    n = min(-c, c0 + ww - c)
                    res.append((c - c0, "L", 0, n))
                elif c >= W:
                    n = c0 + ww - c
                    res.append((c - c0, "R", 0, n))
                else:
                    b, p = divmod(c, 98)
                    n = min(98 - p, c0 + ww - c, W - c)
                    res.append((c - c0, "S", c, n))
                c += n
            return res

        xplan = [pieces(98 * B - HKS, 128) for B in range(NB)]
        splan = [pieces(98 * B - HDS, 102) for B in range(NB)]

        with tile.TileContext(nc) as tc:
            with tc.tile_pool(name="tabs", bufs=1) as tabp, \
                 tc.tile_pool(name="mast", bufs=1) as mast, \
                 tc.tile_pool(name="xw", bufs=NB + 2) as xwp, \
                 tc.tile_pool(name="sw", bufs=4) as swp, \
                 tc.tile_pool(name="zsl", bufs=4) as zslp, \
                 tc.tile_pool(name="edg", bufs=2) as edgp, \
                 tc.tile_pool(name="ps", bufs=4, space="PSUM") as pp, \
                 tc.tile_pool(name="ps2", bufs=4, space="PSUM") as pp2:

                def load_tab(d, p, w_, tag):
                    t = tabp.tile([p, w_], F32R, tag=tag)
                    nc.sync.dma_start(out=t[:, :], in_=d[:, :])
                    return t

                tK = load_tab(tabK_d, 128, KS * 98, "tK")
                tIx = load_tab(tabIx_d, 128, 98, "tIx")
                tG1 = load_tab(tabG1_d, 128, nvG1 * 98, "tG1")
                tG2 = load_tab(tabG2_d, 128, nvG2 * 98, "tG2")
                tD0y = load_tab(tabD0y_d, 102, DS * 98, "tD0y")
                tD1 = load_tab(tabD1_d, 102, DS * 98, "tD1")
                tD2 = load_tab(tabD2_d, 102, DS * 98, "tD2")
                tD0 = load_tab(tabD0_d, 102, DS * 98, "tD0")
                tD1n = load_tab(tabD1n_d, 102, DS * 98, "tD1n")
                tD2n = load_tab(tabD2n_d, 102, DS * 98, "tD2n")
                tIw = load_tab(idw_d, 98, 98, "tIw")
                lam = tabp.tile([98, 1], F32, tag="lam")
                nc.sync.dma_start(out=lam[:, :], in_=lam_d[:, :])
                mtop = tabp.tile([98, 1], F32, tag="mtop")
                nc.sync.dma_start(out=mtop[:, :], in_=mtop_d[:, :])
                mbot = tabp.tile([98, 1], F32, tag="mbot")
                nc.sync.dma_start(out=mbot[:, :], in_=mbot_d[:, :])
                mti = tabp.tile([98, 2], mybir.dt.int32, tag="mti")
                nc.sync.dma_start(out=mti[:, :], in_=mti_d[:, :])
                mbi = tabp.tile([98, 2], mybir.dt.int32, tag="mbi")
                nc.sync.dma_start(out=mbi[:, :], in_=mbi_d[:, :])

                def doubling(t, width):
                    k = 1
                    while k < width:
                        n = min(k, width - k)
                        nc.sync.dma_start(out=t[k:k + n, :], in_=t[0:n, :])
                        k += n

                def edge_from(ap_onecol, width, rows, tag):
                    e = edgp.tile([width, rows], F32R, tag=tag)
                    nc.sync.dma_start(out=e[0:1, :], in_=ap_onecol)
                    doubling(e, width)
                    return e

                I32 = mybir.dt.int32

                def blend2(out_ap, src_1col, imask):
                    nc.vector.copy_predicated(
                        out_ap.bitcast(I32), imask[:, 0:2],
                        src_1col.bitcast(I32).broadcast_to((98, 2)))

                def blend(out_ap, src_1col, mask, tmp_pool_tag, shape):
                    tmp = zslp.tile(list(shape), F32R, tag=tmp_pool_tag)
                    t_ap = tmp[tuple(slice(0, d_) for d_ in shape)]
                    nc.vector.tensor_sub(t_ap, src_1col.broadcast_to(shape),
                                         out_ap)
                    nc.vector.scalar_tensor_tensor(
                        out_ap, t_ap, mask[:, :1], out_ap,
                        mybir.AluOpType.mult, mybir.AluOpType.add)

                def softshrink(dst, src_ap, tmp):
                    nc.vector.tensor_scalar(dst[:, :], src_ap, lam[:, :1],
                                            0.0, mybir.AluOpType.subtract,
                                            mybir.AluOpType.max)
                    nc.vector.tensor_scalar(tmp[:, :], src_ap, lam[:, :1],
                                            0.0, mybir.AluOpType.add,
                                            mybir.AluOpType.min)
                    nc.vector.tensor_add(dst[:, :], dst[:, :], tmp[:, :])

                for ch in range(n_ch):
                    xs = mast.tile([98, NB, BAND], F32R, tag="xs")
                    for b in range(NB):
                        nc.sync.dma_start(
                            out=xs[:, b, :],
                            in_=x_d[ch, 98 * b:98 * b + 98, :])
                    ws = mast.tile([98, NB, LW], F32R, tag="ws")

                    # ---- x0 windows (serve z/w phase AND step 1) ----
                    eLx = edge_from(xs[0:1, 0, :], HKS, BAND, "xeL")
                    eRx = edge_from(xs[lastp:lastp + 1, lastb, :], ERW, BAND,
                                    "xeR")

                    def xwin(B, base, nrows, wb):
                        win = xwp.tile([128, nrows], F32R, tag="xw")
                        for dst, kind, off, n in xplan[B]:
                            if kind == "S":
                                b, p = divmod(off, 98)
                                nc.sync.dma_start(
                                    out=win[dst:dst + n, :],
                                    in_=xs[p:p + n, b,
                                           base:base + nrows])
                            elif kind == "L":
                                nc.sync.dma_start(
                                    out=win[dst:dst + n, :],
                                    in_=eLx[0:n, base:base + nrows]
                                     if wb is None else
                                    wb[0][0:n, base:base + nrows])
                            else:
                                nc.sync.dma_start(
                                    out=win[dst:dst + n, :],
                                    in_=eRx[0:n, base:base + nrows]
                                     if wb is None else
                                    wb[1][0:n, base:base + nrows])
                        return win

                    xw0 = {B: xwin(B, 0, BAND, None) for B in range(NB)}

                    def swin(plan_B, slab_ap, eL, eR, rows, tag):
                        win = swp.tile([102, rows], F32R, tag=tag)
                        for dst, kind, off, n in plan_B:
                            if kind == "S":
                                nc.sync.dma_start(out=win[dst:dst + n, :],
                                                  in_=slab_ap(off, n))
                            elif kind == "L":
                                nc.sync.dma_start(out=win[dst:dst + n, :],
                                                  in_=eL[0:n, :rows])
                            else:
                                nc.sync.dma_start(out=win[dst:dst + n, :],
                                                  in_=eR[0:n, :rows])
                        return win

                    # ======== z/w phase ========
                    z1s, z2s = {}, {}

                    def make_z(B):
                        psz = pp.tile([98, LZ], F32, tag="acc1")
                        for i, v in enumerate(offsG1):
                            nc.tensor.matmul(psz[:, :],
                                             tG1[:, 98 * i:98 * i + 98],
                                             xw0[B][:, zr0 + v:zr0 + v + LZ],
                                             start=(i == 0),
                                             stop=(i == nvG1 - 1))
                        z1 = zslp.tile([98, LZ], F32R, tag="z1")
                        zt = zslp.tile([98, LZ], F32R, tag="zt")
                        softshrink(z1, psz[:, :], zt)
                        psz2 = pp.tile([98, LZ], F32, tag="acc1")
                        for i, v in enumerate(offsG2):
                            nc.tensor.matmul(psz2[:, :],
                                             tG2[:, 98 * i:98 * i + 98],
                                             xw0[B][:, zr0 + v:zr0 + v + LZ],
                                             start=(i == 0),
                                             stop=(i == nvG2 - 1))
                        z2 = zslp.tile([98, LZ], F32R, tag="z2")
                        softshrink(z2, psz2[:, :], zt)
                        for zz in (z1, z2):
                            td = S
                            blend2(zz[:, td:td + 2], zz[:, td + 2:td + 3], mti)
                            bd = BAND - 2 * S - (S - 2)
                            blend2(zz[:, bd:bd + 2], zz[:, bd - 1:bd], mbi)
                        z1s[B], z2s[B] = z1, z2

                    yeL = edge_from(y_d[ch, 0:1, zr0:zr0 + LZ], HDS, LZ,
                                    "yeL")
                    yeR = edge_from(y_d[ch, W - 1:W, zr0:zr0 + LZ], ERS, LZ,
                                    "yeR")
                    make_z(0)
                    ez1L = ez2L = ez1R = ez2R = None
                    for B in range(NB):
                        if B + 1 < NB:
                            make_z(B + 1)
                        if ez1L is None:
                            ez1L = edge_from(z1s[0][0:1, :], HDS, LZ, "z1L")
                            ez2L = edge_from(z2s[0][0:1, :], HDS, LZ, "z2L")
                        if B == NB - 1 and ez1R is None:
                            ez1R = edge_from(z1s[lastb][lastp:lastp + 1, :],
                                             ERS, LZ, "z1R")
                            ez2R = edge_from(z2s[lastb][lastp:lastp + 1, :],
                                             ERS, LZ, "z2R")
                        psw = pp2.tile([98, LW], F32, tag="acc2")
                        yw = swin(splan[B],
                                  lambda off, n: y_d[
                                      ch, off:off + n,
                                      zr0:zr0 + LZ],
                                  yeL, yeR, LZ, "yw")
                        for a in range(DS):
                            nc.tensor.matmul(psw[:, :],
                                             tD0y[:, 98 * a:98 * a + 98],
                                             yw[:, a:a + LW],
                                             start=(a == 0), stop=False)
                        z1w = swin(splan[B],
                                   lambda off, n: z1s[off // 98][
                                       off % 98:off % 98 + n, :],
                                   ez1L, ez1R, LZ, "z1w")
                        for a in range(DS):
                            nc.tensor.matmul(psw[:, :],
                                             tD1[:, 98 * a:98 * a + 98],
                                             z1w[:, a:a + LW],
                                             start=False, stop=False)
                        z2w = swin(splan[B],
                                   lambda off, n: z2s[off // 98][
                                       off % 98:off % 98 + n, :],
                                   ez2L, ez2R, LZ, "z2w")
                        for a in range(DS):
                            nc.tensor.matmul(psw[:, :],
                                             tD2[:, 98 * a:98 * a + 98],
                                             z2w[:, a:a + LW],
                                             start=False, stop=(a == DS - 1))
                        nc.vector.tensor_copy(ws[:, B, :], psw[:, :])
                        z1s.pop(B - 1, None)
                        z2s.pop(B - 1, None)

                    # ======== inner steps ========
                    for step in range(N_IN):
                        r0 = S * (step + 1)
                        L = BAND - 2 * r0
                        if step == 0:
                            wbase, xws = 0, xw0
                        else:
                            wbase = r0 - S
                            eLx2 = edge_from(xs[0:1, 0, :], HKS, BAND, "xeL")
                            eRx2 = edge_from(xs[lastp:lastp + 1, lastb, :],
                                             ERW, BAND, "xeR")
                            xws = {B: xwin(B, wbase, L + 4 + 2 * HKS,
                                           (eLx2, eRx2)) for B in range(NB)}
                        kxs = {}
                        g1s, g2s = {}, {}
                        ekxL = ekxR = None
                        eg = {}

                        def make_g(B):
                            for (tG, nvG, offsG, dct, tag) in (
                                    (tG1, nvG1, offsG1, g1s, "g1"),
                                    (tG2, nvG2, offsG2, g2s, "g2")):
                                psg = pp.tile([98, L + 4], F32, tag="acc1")
                                for i, v in enumerate(offsG):
                                    o = r0 - 2 + v - wbase
                                    nc.tensor.matmul(
                                        psg[:, :], tG[:, 98 * i:98 * i + 98],
                                        xws[B][:, o:o + L + 4],
                                        start=(i == 0), stop=(i == nvG - 1))
                                g = zslp.tile([98, L + 4], F32R, tag=tag)
                                nc.vector.tensor_copy(g[:, :], psg[:, :])
                                td = 2 * S - r0
                                blend2(g[:, td:td + 2], g[:, td + 2:td + 3],
                                       mti)
                                bd = BAND - 2 * S - (r0 - 2)
                                blend2(g[:, bd:bd + 2], g[:, bd - 1:bd], mbi)
                                dct[B] = g

                        def make_kx(B):
                            psk = pp.tile([98, L + 4], F32, tag="acc1")
                            for a in range(KS):
                                o = a + r0 - wbase - HKS - HDS
                                nc.tensor.matmul(psk[:, :],
                                                 tK[:, 98 * a:98 * a + 98],
                                                 xws[B][:, o:o + L + 4],
                                                 start=(a == 0),
                                                 stop=(a == KS - 1))
                            kx = zslp.tile([98, L + 4], F32R, tag="kx")
                            nc.vector.tensor_copy(kx[:, :], psk[:, :])
                            # vertical replicate blend: global rows -2,-1 <-
                            # row 0 (core 0); rows H, H+1 <- H-1 (last core)
                            td = 2 * S - r0
                            blend2(kx[:, td:td + 2], kx[:, td + 2:td + 3],
                                   mti)
                            bd = BAND - 2 * S - (r0 - 2)
                            blend2(kx[:, bd:bd + 2], kx[:, bd - 1:bd], mbi)
                            kxs[B] = kx

                        make_kx(0)
                        make_g(0)
                        for B in range(NB):
                            if B + 1 < NB:
                                make_kx(B + 1)
                                make_g(B + 1)
                            if ekxL is None:
                                ekxL = edge_from(kxs[0][0:1, :], HDS, L + 4,
                                                 "kxL")
                                eg["g1L"] = edge_from(g1s[0][0:1, :], HDS,
                                                      L + 4, "g1L")
                                eg["g2L"] = edge_from(g2s[0][0:1, :], HDS,
                                                      L + 4, "g2L")
                            if B == NB - 1 and ekxR is None:
                                ekxR = edge_from(
                                    kxs[lastb][lastp:lastp + 1, :], ERS,
                                    L + 4, "kxR")
                                eg["g1R"] = edge_from(
                                    g1s[lastb][lastp:lastp + 1, :], ERS,
                                    L + 4, "g1R")
                                eg["g2R"] = edge_from(
                                    g2s[lastb][lastp:lastp + 1, :], ERS,
                                    L + 4, "g2R")
                            psx = pp2.tile([98, L], F32, tag="acc2")
                            nc.tensor.matmul(
                                psx[:, :], tIw[:, :],
                                ws[:, B, r0 - S:r0 - S + L],
                                start=True, stop=False)
                            nc.tensor.matmul(psx[:, :], tIx[:, :98],
                                             xws[B][:, r0 - wbase:
                                                    r0 - wbase + L],
                                             start=False, stop=False)
                            kw = swin(splan[B],
                                      lambda off, n: kxs[off // 98][
                                          off % 98:off % 98 + n, :],
                                      ekxL, ekxR, L + 4, "kw")
                            for a in range(DS):
                                nc.tensor.matmul(psx[:, :],
                                                 tD0[:, 98 * a:98 * a + 98],
                                                 kw[:, a:a + L],
                                                 start=False, stop=False)
                            g1w = swin(splan[B],
                                       lambda off, n: g1s[off // 98][
                                           off % 98:off % 98 + n, :],
                                       eg.get("g1L"), eg.get("g1R"), L + 4, "g1w")
                            for a in range(DS):
                                nc.tensor.matmul(psx[:, :],
                                                 tD1n[:, 98 * a:98 * a + 98],
                                                 g1w[:, a:a + L],
                                                 start=False, stop=False)
                            g2w = swin(splan[B],
                                       lambda off, n: g2s[off // 98][
                                           off % 98:off % 98 + n, :],
                                       eg.get("g2L"), eg.get("g2R"), L + 4, "g2w")
                            for a in range(DS):
                                nc.tensor.matmul(psx[:, :],
                                                 tD2n[:, 98 * a:98 * a + 98],
                                                 g2w[:, a:a + L],
                                                 start=False,
                                                 stop=(a == DS - 1))
                            if step == N_IN - 1:
                                nc.vector.tensor_scalar(
                                    xs[:, B, r0:r0 + L], psx[:, :], 0.0, 1.0,
                                    mybir.AluOpType.max, mybir.AluOpType.min)
                            else:
                                nc.vector.tensor_copy(xs[:, B, r0:r0 + L],
                                                      psx[:, :])
                            kxs.pop(B - 1, None)
                            g1s.pop(B - 1, None)
                            g2s.pop(B - 1, None)
                        if step < N_IN - 1:
                            blend(xs[:, :, S:2 * S],
                                  xs[:, :, 2 * S:2 * S + 1], mtop,
                                  "btmp3", (98, NB, S))
                            blend(xs[:, :, BAND - 2 * S:BAND - S],
                                  xs[:, :, BAND - 2 * S - 1:BAND - 2 * S],
                                  mbot, "btmp3", (98, NB, S))

                    for b in range(NB):
                        nc.sync.dma_start(
                            out=out_d[ch, 98 * b:98 * b + 98, :],
                            in_=xs[:, b, 2 * S:2 * S + self.OWN].bitcast(F32))

        nc.compile()
        return nc


LAST_EXEC_NS = None


def run_chqs(input_img, k, d, weight, n_cores=8, runner=None, trace=False):
    B0, C, H, W = input_img.shape
    OWN = H // n_cores
    k2d = np.asarray(k, np.float32)[0, 0]
    d = np.asarray(d, np.float32)
    weight = np.asarray(weight, np.float32)
    offsG1 = _g_offsets(weight[0, 0])
    offsG2 = _g_offsets(weight[1, 0])
    bld = Builder(W, OWN, C, n_cores, offsG1, offsG2)
    nc = bld.build()
    NB, WPAD = bld.NB, bld.WPAD

    img = np.asarray(input_img, np.float32)[0]

    def to_planes(a):
        t = np.transpose(a, (0, 2, 1))
        if WPAD > W:
            t = np.concatenate(
                [t, np.repeat(t[:, W - 1:W, :], WPAD - W, axis=1)], axis=1)
        return np.ascontiguousarray(t)

    y_pl = to_planes(img)

    def band_of(pl, c):
        idx = np.clip(np.arange(OWN * c - 2 * S, OWN * c + OWN + 2 * S),
                      0, H - 1)
        return np.ascontiguousarray(pl[:, :, idx])

    y_bands = [band_of(y_pl, c) for c in range(n_cores)]
    mt = [np.full((98, 1), 1.0 if c == 0 else 0.0, np.float32)
          for c in range(n_cores)]
    mb = [np.full((98, 1), 1.0 if c == n_cores - 1 else 0.0, np.float32)
          for c in range(n_cores)]
    mti = [np.full((98, 2), 1 if c == 0 else 0, np.int32)
           for c in range(n_cores)]
    mbi = [np.full((98, 2), 1 if c == n_cores - 1 else 0, np.int32)
           for c in range(n_cores)]

    x_pl = y_pl.copy()
    for it in range(N_ITER):
        lamv = LAMBD / max(1e-4, float(BETA[it]))
        tabs = make_tables(k2d, d[it], weight, offsG1, offsG2)
        in_maps = []
        for c in range(n_cores):
            m = dict(tabs)
            m["x"] = band_of(x_pl, c)
            m["y"] = y_bands[c]
            m["lam"] = np.full((98, 1), lamv, np.float32)
            m["mtop"] = mt[c]
            m["mbot"] = mb[c]
            m["mti"] = mti[c]
            m["mbi"] = mbi[c]
            in_maps.append(m)
        if runner is None:
            res = run_bass_kernel_spmd(nc, in_maps, list(range(n_cores)),
                                       trace=trace)
            outs = res.results
            if res.exec_time_ns:
                global LAST_EXEC_NS
                LAST_EXEC_NS = (LAST_EXEC_NS or 0) + res.exec_time_ns
        else:
            outs = runner(nc, in_maps)
        for c in range(n_cores):
            x_pl[:, :, OWN * c:OWN * c + OWN] = outs[c]["o"]
    return np.ascontiguousarray(
        np.transpose(x_pl[:, :W, :], (0, 2, 1)))[None].astype(np.float32)


def kernel(input, k, d, weight):
    return run_chqs(input, k, d, weight, n_cores=8)


# revision 14
# speedup vs baseline: 1.2878x; 1.2855x over previous
"""CHQS deconvolution kernel for Trainium2 (8 NeuronCores).

Reference computation: 5 outer iterations of
  z = softshrink(G x, lam_i); then 2x { r0 = y - Kx; r1 = z - Gx;
  x += D_i * pad([r0, r1], 2) }; x = clip(x, 0, 1)
with K a 31x31 blur (replicate pad 15), G 2-channel finite-diff (pad 2),
D_i [3,5,5] (pad 2).

Implementation identity (replicate-pad is linear: pad(a-b) = pad(a)-pad(b)):
  x_new = x + w - D0*pad(Kx) - CC*x
  w  = D0*pad(y) + D1*pad(z1) + D2*pad(z2)    (per outer iteration)
  CC = D1*G1 + D2*G2                          (composed 9x9 conv)
All convs run on the tensor engine as per-kernel-row Toeplitz matmuls in
float32r (full-rate, ~1e-4 accurate). Image layout transposed:
[cols -> partitions, rows -> free]; 98-col slabs; conv inputs are
DMA-built 128/102-col windows. Horizontal replicate-pad = replicated
edge-col tiles; vertical replicate-pad = maintained pad rows (masked so
only the true image top/bottom cores blend).

Sharding: 8 cores x (H/8) output rows, one launch per outer iteration
(one compiled program; d_i-dependent Toeplitz tables are inputs).
"""

import math
import numpy as np

import concourse.bacc as bacc
import concourse.mybir as mybir
import concourse.tile as tile
from concourse.bass_utils import run_bass_kernel_spmd

F32 = mybir.dt.float32
F32R = mybir.dt.float32r

N_ITER = 5
N_IN = 2
LAMBD = 0.005
BETA = (np.array([0.0, 1.0, 4.0, 16.0, 64.0, 256.0, 1024.0, 4096.0,
                  16384.0, 65536.0]) * 0.001 / 10.0 * 81.0)

KS, HKS = 31, 15
DS, HDS = 5, 2
S = HKS + HDS  # 17 halo rows consumed per inner step


def _toeplitz(rows, win_w, out_w, pad, R):
    A, T = rows.shape
    tabs = np.zeros((A, win_w, out_w), dtype=np.float32)
    for j in range(out_w):
        for t in range(T):
            c = j + pad + t - R
            if 0 <= c < win_w:
                tabs[:, c, j] = rows[:, t]
    return tabs


def _flat(tabs):
    A, P, O = tabs.shape
    return np.ascontiguousarray(tabs.transpose(1, 0, 2)).reshape(P, A * O)


def _g_offsets(g):
    nz = [a for a in range(DS) if np.any(g[a] != 0)]
    return [a - HDS for a in (nz or [HDS])]


def _cc_offsets(weight):
    offs = {0}
    for chn in (0, 1):
        for a in range(DS):
            if np.any(weight[chn, 0, a] != 0):
                for p in range(DS):
                    offs.add((p - HDS) + (a - HDS))
    return sorted(offs)


def make_tables(k2d, d_i, weight, offsG1, offsG2):
    tabK = _flat(_toeplitz(k2d, 128, 98, HKS, HKS))
    g1, g2 = weight[0, 0], weight[1, 0]
    tG1 = _flat(_toeplitz(np.stack([g1[v + HDS] for v in offsG1]),
                          128, 98, HKS, HDS))
    tG2 = _flat(_toeplitz(np.stack([g2[v + HDS] for v in offsG2]),
                          128, 98, HKS, HDS))
    tIx = _flat(_toeplitz(np.ones((1, 1), np.float32), 128, 98, HKS, 0))
    return dict(
        tabK=tabK, tabIx=tIx, tabG1=tG1, tabG2=tG2,
        tabD0y=_flat(_toeplitz(d_i[0], 102, 98, HDS, HDS)),
        tabD1=_flat(_toeplitz(d_i[1], 102, 98, HDS, HDS)),
        tabD2=_flat(_toeplitz(d_i[2], 102, 98, HDS, HDS)),
        tabD0=_flat(_toeplitz(-d_i[0], 102, 98, HDS, HDS)),
        tabD1n=_flat(_toeplitz(-d_i[1], 102, 98, HDS, HDS)),
        tabD2n=_flat(_toeplitz(-d_i[2], 102, 98, HDS, HDS)),
        idw=np.eye(98, dtype=np.float32))


class Builder:
    def __init__(self, W, OWN, n_ch, n_cores, offsG1, offsG2):
        self.W, self.OWN, self.n_ch, self.n_cores = W, OWN, n_ch, n_cores
        self.NB = math.ceil(W / 98)
        self.WPAD = self.NB * 98
        self.BAND = OWN + 4 * S
        self.L1 = OWN + 2 * S
        self.LW = self.L1
        self.LZ = self.LW + 2 * HDS
        self.offsG1, self.offsG2 = offsG1, offsG2
        self.fake = self.WPAD - W
        self.ERW = HKS + self.fake
        self.ERS = HDS + self.fake

    def build(self):
        W, NB, BAND, n_ch = self.W, self.NB, self.BAND, self.n_ch
        LW, LZ = self.LW, self.LZ
        offsG1, offsG2 = self.offsG1, self.offsG2
        nvG1, nvG2 = len(offsG1), len(offsG2)
        ERW, ERS = self.ERW, self.ERS
        zr0 = S - HDS
        lastb, lastp = divmod(W - 1, 98)

        nc = bacc.Bacc("TRN2", target_bir_lowering=False, debug=False,
                       num_devices=self.n_cores)
        din = lambda n, s, dt=F32R: nc.dram_tensor(
            n, s, dt, kind="ExternalInput").ap()
        x_d = din("x", (n_ch, self.WPAD, BAND))
        y_d = din("y", (n_ch, self.WPAD, BAND))
        tabK_d = din("tabK", (128, KS * 98))
        tabIx_d = din("tabIx", (128, 98))
        tabG1_d = din("tabG1", (128, nvG1 * 98))
        tabG2_d = din("tabG2", (128, nvG2 * 98))
        tabD0y_d = din("tabD0y", (102, DS * 98))
        tabD1_d = din("tabD1", (102, DS * 98))
        tabD2_d = din("tabD2", (102, DS * 98))
        tabD0_d = din("tabD0", (102, DS * 98))
        tabD1n_d = din("tabD1n", (102, DS * 98))
        tabD2n_d = din("tabD2n", (102, DS * 98))
        idw_d = din("idw", (98, 98))
        lam_d = din("lam", (98, 1), F32)
        mtop_d = din("mtop", (98, 1), F32)
        mbot_d = din("mbot", (98, 1), F32)
        mti_d = din("mti", (98, 2), mybir.dt.int32)
        mbi_d = din("mbi", (98, 2), mybir.dt.int32)
        out_d = nc.dram_tensor("o", (n_ch, self.WPAD, self.OWN), F32,
                               kind="ExternalOutput").ap()

        def pieces(c0, ww):
            res, c = [], c0
            while c < c0 + ww:
                if c < 0:
                    n = min(-c, c0 + ww - c)
                    res.append((c - c0, "L", 0, n))
                elif c >= W:
                    n = c0 + ww - c
                    res.append((c - c0, "R", 0, n))
                else:
                    b, p = divmod(c, 98)
                    n = min(98 - p, c0 + ww - c, W - c)
                    res.append((c - c0, "S", c, n))
                c += n
            return res

        xplan = [pieces(98 * B - HKS, 128) for B in range(NB)]
        splan = [pieces(98 * B - HDS, 102) for B in range(NB)]

        with tile.TileContext(nc) as tc:
            with tc.tile_pool(name="tabs", bufs=1) as tabp, \
                 tc.tile_pool(name="mast", bufs=1) as mast, \
                 tc.tile_pool(name="xw", bufs=NB + 2) as xwp, \
                 tc.tile_pool(name="sw", bufs=4) as swp, \
                 tc.tile_pool(name="zsl", bufs=4) as zslp, \
                 tc.tile_pool(name="edg", bufs=2) as edgp, \
                 tc.tile_pool(name="ps", bufs=4, space="PSUM") as pp, \
                 tc.tile_pool(name="ps2", bufs=4, space="PSUM") as pp2:

                _dmaeng = [nc.sync, nc.gpsimd, nc.scalar]
                _dmaidx = [0]

                def dma(out, in_):
                    e = _dmaeng[_dmaidx[0] % 3]
                    _dmaidx[0] += 1
                    e.dma_start(out=out, in_=in_)

                def load_tab(d, p, w_, tag):
                    t = tabp.tile([p, w_], F32R, tag=tag)
                    dma(out=t[:, :], in_=d[:, :])
                    return t

                tK = load_tab(tabK_d, 128, KS * 98, "tK")
                tIx = load_tab(tabIx_d, 128, 98, "tIx")
                tG1 = load_tab(tabG1_d, 128, nvG1 * 98, "tG1")
                tG2 = load_tab(tabG2_d, 128, nvG2 * 98, "tG2")
                tD0y = load_tab(tabD0y_d, 102, DS * 98, "tD0y")
                tD1 = load_tab(tabD1_d, 102, DS * 98, "tD1")
                tD2 = load_tab(tabD2_d, 102, DS * 98, "tD2")
                tD0 = load_tab(tabD0_d, 102, DS * 98, "tD0")
                tD1n = load_tab(tabD1n_d, 102, DS * 98, "tD1n")
                tD2n = load_tab(tabD2n_d, 102, DS * 98, "tD2n")
                tIw = load_tab(idw_d, 98, 98, "tIw")
                lam = tabp.tile([98, 1], F32, tag="lam")
                dma(out=lam[:, :], in_=lam_d[:, :])
                mtop = tabp.tile([98, 1], F32, tag="mtop")
                dma(out=mtop[:, :], in_=mtop_d[:, :])
                mbot = tabp.tile([98, 1], F32, tag="mbot")
                dma(out=mbot[:, :], in_=mbot_d[:, :])
                mti = tabp.tile([98, 2], mybir.dt.int32, tag="mti")
                dma(out=mti[:, :], in_=mti_d[:, :])
                mbi = tabp.tile([98, 2], mybir.dt.int32, tag="mbi")
                dma(out=mbi[:, :], in_=mbi_d[:, :])

                def doubling(t, width):
                    k = 1
                    while k < width:
                        n = min(k, width - k)
                        dma(out=t[k:k + n, :], in_=t[0:n, :])
                        k += n

                def edge_from(ap_onecol, width, rows, tag):
                    e = edgp.tile([width, rows], F32R, tag=tag)
                    dma(out=e[0:1, :], in_=ap_onecol)
                    doubling(e, width)
                    return e

                I32 = mybir.dt.int32

                def blend2(out_ap, src_1col, imask):
                    nc.vector.copy_predicated(
                        out_ap.bitcast(I32), imask[:, 0:2],
                        src_1col.bitcast(I32).broadcast_to((98, 2)))

                def blend(out_ap, src_1col, mask, tmp_pool_tag, shape):
                    tmp = zslp.tile(list(shape), F32R, tag=tmp_pool_tag)
                    t_ap = tmp[tuple(slice(0, d_) for d_ in shape)]
                    nc.vector.tensor_sub(t_ap, src_1col.broadcast_to(shape),
                                         out_ap)
                    nc.vector.scalar_tensor_tensor(
                        out_ap, t_ap, mask[:, :1], out_ap,
                        mybir.AluOpType.mult, mybir.AluOpType.add)

                def softshrink(dst, src_ap, tmp):
                    nc.vector.tensor_scalar(dst[:, :], src_ap, lam[:, :1],
                                            0.0, mybir.AluOpType.subtract,
                                            mybir.AluOpType.max)
                    nc.vector.tensor_scalar(tmp[:, :], src_ap, lam[:, :1],
                                            0.0, mybir.AluOpType.add,
                                            mybir.AluOpType.min)
                    nc.vector.tensor_add(dst[:, :], dst[:, :], tmp[:, :])

                for ch in range(n_ch):
                    xs = mast.tile([98, NB, BAND], F32R, tag="xs")
                    for b in range(NB):
                        nc.sync.dma_start(
                            out=xs[:, b, :],
                            in_=x_d[ch, 98 * b:98 * b + 98, :])
                    ws = mast.tile([98, NB, LW], F32R, tag="ws")

                    # ---- x0 windows (serve z/w phase AND step 1) ----
                    eLx = edge_from(xs[0:1, 0, :], HKS, BAND, "xeL")
                    eRx = edge_from(xs[lastp:lastp + 1, lastb, :], ERW, BAND,
                                    "xeR")

                    def xwin(B, base, nrows, wb):
                        win = xwp.tile([128, nrows], F32R, tag="xw")
                        for dst, kind, off, n in xplan[B]:
                            if kind == "S":
                                b, p = divmod(off, 98)
                                nc.sync.dma_start(
                                    out=win[dst:dst + n, :],
                                    in_=xs[p:p + n, b,
                                           base:base + nrows])
                            elif kind == "L":
                                nc.sync.dma_start(
                                    out=win[dst:dst + n, :],
                                    in_=eLx[0:n, base:base + nrows]
                                     if wb is None else
                                    wb[0][0:n, base:base + nrows])
                            else:
                                nc.sync.dma_start(
                                    out=win[dst:dst + n, :],
                                    in_=eRx[0:n, base:base + nrows]
                                     if wb is None else
                                    wb[1][0:n, base:base + nrows])
                        return win

                    xw0 = {B: xwin(B, 0, BAND, None) for B in range(NB)}

                    def swin(plan_B, slab_ap, eL, eR, rows, tag):
                        win = swp.tile([102, rows], F32R, tag=tag)
                        for dst, kind, off, n in plan_B:
                            if kind == "S":
                                dma(out=win[dst:dst + n, :],
                                                  in_=slab_ap(off, n))
                            elif kind == "L":
                                dma(out=win[dst:dst + n, :],
                                                  in_=eL[0:n, :rows])
                            else:
                                dma(out=win[dst:dst + n, :],
                                                  in_=eR[0:n, :rows])
                        return win

                    # ======== z/w phase ========
                    z1s, z2s = {}, {}

                    def make_z(B):
                        psz = pp.tile([98, LZ], F32, tag="acc1")
                        for i, v in enumerate(offsG1):
                            nc.tensor.matmul(psz[:, :],
                                             tG1[:, 98 * i:98 * i + 98],
                                             xw0[B][:, zr0 + v:zr0 + v + LZ],
                                             start=(i == 0),
                                             stop=(i == nvG1 - 1))
                        z1 = zslp.tile([98, LZ], F32R, tag="z1")
                        zt = zslp.tile([98, LZ], F32R, tag="zt")
                        softshrink(z1, psz[:, :], zt)
                        psz2 = pp.tile([98, LZ], F32, tag="acc1")
                        for i, v in enumerate(offsG2):
                            nc.tensor.matmul(psz2[:, :],
                                             tG2[:, 98 * i:98 * i + 98],
                                             xw0[B][:, zr0 + v:zr0 + v + LZ],
                                             start=(i == 0),
                                             stop=(i == nvG2 - 1))
                        z2 = zslp.tile([98, LZ], F32R, tag="z2")
                        softshrink(z2, psz2[:, :], zt)
                        for zz in (z1, z2):
                            td = S
                            blend2(zz[:, td:td + 2], zz[:, td + 2:td + 3], mti)
                            bd = BAND - 2 * S - (S - 2)
                            blend2(zz[:, bd:bd + 2], zz[:, bd - 1:bd], mbi)
                        z1s[B], z2s[B] = z1, z2

                    yeL = edge_from(y_d[ch, 0:1, zr0:zr0 + LZ], HDS, LZ,
                                    "yeL")
                    yeR = edge_from(y_d[ch, W - 1:W, zr0:zr0 + LZ], ERS, LZ,
                                    "yeR")
                    make_z(0)
                    ez1L = ez2L = ez1R = ez2R = None
                    for B in range(NB):
                        if B + 1 < NB:
                            make_z(B + 1)
                        if ez1L is None:
                            ez1L = edge_from(z1s[0][0:1, :], HDS, LZ, "z1L")
                            ez2L = edge_from(z2s[0][0:1, :], HDS, LZ, "z2L")
                        if B == NB - 1 and ez1R is None:
                            ez1R = edge_from(z1s[lastb][lastp:lastp + 1, :],
                                             ERS, LZ, "z1R")
                            ez2R = edge_from(z2s[lastb][lastp:lastp + 1, :],
                                             ERS, LZ, "z2R")
                        psw = pp2.tile([98, LW], F32, tag="acc2")
                        yw = swin(splan[B],
                                  lambda off, n: y_d[
                                      ch, off:off + n,
                                      zr0:zr0 + LZ],
                                  yeL, yeR, LZ, "yw")
                        for a in range(DS):
                            nc.tensor.matmul(psw[:, :],
                                             tD0y[:, 98 * a:98 * a + 98],
                                             yw[:, a:a + LW],
                                             start=(a == 0), stop=False)
                        z1w = swin(splan[B],
                                   lambda off, n: z1s[off // 98][
                                       off % 98:off % 98 + n, :],
                                   ez1L, ez1R, LZ, "z1w")
                        for a in range(DS):
                            nc.tensor.matmul(psw[:, :],
                                             tD1[:, 98 * a:98 * a + 98],
                                             z1w[:, a:a + LW],
                                             start=False, stop=False)
                        z2w = swin(splan[B],
                                   lambda off, n: z2s[off // 98][
                                       off % 98:off % 98 + n, :],
                                   ez2L, ez2R, LZ, "z2w")
                        for a in range(DS):
                            nc.tensor.matmul(psw[:, :],
                                             tD2[:, 98 * a:98 * a + 98],
                                             z2w[:, a:a + LW],
                                             start=False, stop=(a == DS - 1))
                        nc.vector.tensor_copy(ws[:, B, :], psw[:, :])
                        z1s.pop(B - 1, None)
                        z2s.pop(B - 1, None)

                    # ======== inner steps ========
                    for step in range(N_IN):
                        r0 = S * (step + 1)
                        L = BAND - 2 * r0
                        if step == 0:
                            wbase, xws = 0, xw0
                        else:
                            wbase = r0 - S
                            eLx2 = edge_from(xs[0:1, 0, :], HKS, BAND, "xeL")
                            eRx2 = edge_from(xs[lastp:lastp + 1, lastb, :],
                                             ERW, BAND, "xeR")
                            xws = {B: xwin(B, wbase, L + 4 + 2 * HKS,
                                           (eLx2, eRx2)) for B in range(NB)}
                        kxs = {}
                        g1s, g2s = {}, {}
                        ekxL = ekxR = None
                        eg = {}

                        def make_g(B):
                            for (tG, nvG, offsG, dct, tag) in (
                                    (tG1, nvG1, offsG1, g1s, "g1"),
                                    (tG2, nvG2, offsG2, g2s, "g2")):
                                psg = pp.tile([98, L + 4], F32, tag="acc1")
                                for i, v in enumerate(offsG):
                                    o = r0 - 2 + v - wbase
                                    nc.tensor.matmul(
                                        psg[:, :], tG[:, 98 * i:98 * i + 98],
                                        xws[B][:, o:o + L + 4],
                                        start=(i == 0), stop=(i == nvG - 1))
                                g = zslp.tile([98, L + 4], F32R, tag=tag)
                                nc.vector.tensor_copy(g[:, :], psg[:, :])
                                td = 2 * S - r0
                                blend2(g[:, td:td + 2], g[:, td + 2:td + 3],
                                       mti)
                                bd = BAND - 2 * S - (r0 - 2)
                                blend2(g[:, bd:bd + 2], g[:, bd - 1:bd], mbi)
                                dct[B] = g

                        def make_kx(B):
                            psk = pp.tile([98, L + 4], F32, tag="acc1")
                            for a in range(KS):
                                o = a + r0 - wbase - HKS - HDS
                                nc.tensor.matmul(psk[:, :],
                                                 tK[:, 98 * a:98 * a + 98],
                                                 xws[B][:, o:o + L + 4],
                                                 start=(a == 0),
                                                 stop=(a == KS - 1))
                            kx = zslp.tile([98, L + 4], F32R, tag="kx")
                            nc.vector.tensor_copy(kx[:, :], psk[:, :])
                            # vertical replicate blend: global rows -2,-1 <-
                            # row 0 (core 0); rows H, H+1 <- H-1 (last core)
                            td = 2 * S - r0
                            blend2(kx[:, td:td + 2], kx[:, td + 2:td + 3],
                                   mti)
                            bd = BAND - 2 * S - (r0 - 2)
                            blend2(kx[:, bd:bd + 2], kx[:, bd - 1:bd], mbi)
                            kxs[B] = kx

                        make_kx(0)
                        make_g(0)
                        for B in range(NB):
                            if B + 1 < NB:
                                make_kx(B + 1)
                                make_g(B + 1)
                            if ekxL is None:
                                ekxL = edge_from(kxs[0][0:1, :], HDS, L + 4,
                                                 "kxL")
                                eg["g1L"] = edge_from(g1s[0][0:1, :], HDS,
                                                      L + 4, "g1L")
                                eg["g2L"] = edge_from(g2s[0][0:1, :], HDS,
                                                      L + 4, "g2L")
                            if B == NB - 1 and ekxR is None:
                                ekxR = edge_from(
                                    kxs[lastb][lastp:lastp + 1, :], ERS,
                                    L + 4, "kxR")
                                eg["g1R"] = edge_from(
                                    g1s[lastb][lastp:lastp + 1, :], ERS,
                                    L + 4, "g1R")
                                eg["g2R"] = edge_from(
                                    g2s[lastb][lastp:lastp + 1, :], ERS,
                                    L + 4, "g2R")
                            psx = pp2.tile([98, L], F32, tag="acc2")
                            nc.tensor.matmul(
                                psx[:, :], tIw[:, :],
                                ws[:, B, r0 - S:r0 - S + L],
                                start=True, stop=False)
                            nc.tensor.matmul(psx[:, :], tIx[:, :98],
                                             xws[B][:, r0 - wbase:
                                                    r0 - wbase + L],
                                             start=False, stop=False)
                            kw = swin(splan[B],
                                      lambda off, n: kxs[off // 98][
                                          off % 98:off % 98 + n, :],
                                      ekxL, ekxR, L + 4, "kw")
                            for a in range(DS):
                                nc.tensor.matmul(psx[:, :],
                                                 tD0[:, 98 * a:98 * a + 98],
                                                 kw[:, a:a + L],
                                                 start=False, stop=False)
                            g1w = swin(splan[B],
                                       lambda off, n: g1s[off // 98][
                                           off % 98:off % 98 + n, :],
                                       eg.get("g1L"), eg.get("g1R"), L + 4, "g1w")
                            for a in range(DS):
                                nc.tensor.matmul(psx[:, :],
                                                 tD1n[:, 98 * a:98 * a + 98],
                                                 g1w[:, a:a + L],
                                                 start=False, stop=False)
                            g2w = swin(splan[B],
                                       lambda off, n: g2s[off // 98][
                                           off % 98:off % 98 + n, :],
                                       eg.get("g2L"), eg.get("g2R"), L + 4, "g2w")
                            for a in range(DS):
                                nc.tensor.matmul(psx[:, :],
                                                 tD2n[:, 98 * a:98 * a + 98],
                                                 g2w[:, a:a + L],
                                                 start=False,
                                                 stop=(a == DS - 1))
                            if step == N_IN - 1:
                                nc.vector.tensor_scalar(
                                    xs[:, B, r0:r0 + L], psx[:, :], 0.0, 1.0,
                                    mybir.AluOpType.max, mybir.AluOpType.min)
                            else:
                                nc.vector.tensor_copy(xs[:, B, r0:r0 + L],
                                                      psx[:, :])
                            kxs.pop(B - 1, None)
                            g1s.pop(B - 1, None)
                            g2s.pop(B - 1, None)
                        if step < N_IN - 1:
                            blend(xs[:, :, S:2 * S],
                                  xs[:, :, 2 * S:2 * S + 1], mtop,
                                  "btmp3", (98, NB, S))
                            blend(xs[:, :, BAND - 2 * S:BAND - S],
                                  xs[:, :, BAND - 2 * S - 1:BAND - 2 * S],
                                  mbot, "btmp3", (98, NB, S))

                    for b in range(NB):
                        nc.sync.dma_start(
                            out=out_d[ch, 98 * b:98 * b + 98, :],
                            in_=xs[:, b, 2 * S:2 * S + self.OWN].bitcast(F32))

        nc.compile()
        return nc


LAST_EXEC_NS = None


def run_chqs(input_img, k, d, weight, n_cores=8, runner=None, trace=False):
    B0, C, H, W = input_img.shape
    OWN = H // n_cores
    k2d = np.asarray(k, np.float32)[0, 0]
    d = np.asarray(d, np.float32)
    weight = np.asarray(weight, np.float32)
    offsG1 = _g_offsets(weight[0, 0])
    offsG2 = _g_offsets(weight[1, 0])
    bld = Builder(W, OWN, C, n_cores, offsG1, offsG2)
    nc = bld.build()
    NB, WPAD = bld.NB, bld.WPAD

    img = np.asarray(input_img, np.float32)[0]

    def to_planes(a):
        t = np.transpose(a, (0, 2, 1))
        if WPAD > W:
            t = np.concatenate(
                [t, np.repeat(t[:, W - 1:W, :], WPAD - W, axis=1)], axis=1)
        return np.ascontiguousarray(t)

    y_pl = to_planes(img)

    def band_of(pl, c):
        idx = np.clip(np.arange(OWN * c - 2 * S, OWN * c + OWN + 2 * S),
                      0, H - 1)
        return np.ascontiguousarray(pl[:, :, idx])

    y_bands = [band_of(y_pl, c) for c in range(n_cores)]
    mt = [np.full((98, 1), 1.0 if c == 0 else 0.0, np.float32)
          for c in range(n_cores)]
    mb = [np.full((98, 1), 1.0 if c == n_cores - 1 else 0.0, np.float32)
          for c in range(n_cores)]
    mti = [np.full((98, 2), 1 if c == 0 else 0, np.int32)
           for c in range(n_cores)]
    mbi = [np.full((98, 2), 1 if c == n_cores - 1 else 0, np.int32)
           for c in range(n_cores)]

    x_pl = y_pl.copy()
    for it in range(N_ITER):
        lamv = LAMBD / max(1e-4, float(BETA[it]))
        tabs = make_tables(k2d, d[it], weight, offsG1, offsG2)
        in_maps = []
        for c in range(n_cores):
            m = dict(tabs)
            m["x"] = band_of(x_pl, c)
            m["y"] = y_bands[c]
            m["lam"] = np.full((98, 1), lamv, np.float32)
            m["mtop"] = mt[c]
            m["mbot"] = mb[c]
            m["mti"] = mti[c]
            m["mbi"] = mbi[c]
            in_maps.append(m)
        if runner is None:
            res = run_bass_kernel_spmd(nc, in_maps, list(range(n_cores)),
                                       trace=trace)
            outs = res.results
            if res.exec_time_ns:
                global LAST_EXEC_NS
                LAST_EXEC_NS = (LAST_EXEC_NS or 0) + res.exec_time_ns
        else:
            outs = runner(nc, in_maps)
        for c in range(n_cores):
            x_pl[:, :, OWN * c:OWN * c + OWN] = outs[c]["o"]
    return np.ascontiguousarray(
        np.transpose(x_pl[:, :W, :], (0, 2, 1)))[None].astype(np.float32)


def kernel(input, k, d, weight):
    return run_chqs(input, k, d, weight, n_cores=8)


# revision 15
# speedup vs baseline: 1.3080x; 1.0157x over previous
"""CHQS deconvolution kernel for Trainium2 (8 NeuronCores).

Reference computation: 5 outer iterations of
  z = softshrink(G x, lam_i); then 2x { r0 = y - Kx; r1 = z - Gx;
  x += D_i * pad([r0, r1], 2) }; x = clip(x, 0, 1)
with K a 31x31 blur (replicate pad 15), G 2-channel finite-diff (pad 2),
D_i [3,5,5] (pad 2).

Implementation identity (replicate-pad is linear: pad(a-b) = pad(a)-pad(b)):
  x_new = x + w - D0*pad(Kx) - CC*x
  w  = D0*pad(y) + D1*pad(z1) + D2*pad(z2)    (per outer iteration)
  CC = D1*G1 + D2*G2                          (composed 9x9 conv)
All convs run on the tensor engine as per-kernel-row Toeplitz matmuls in
float32r (full-rate, ~1e-4 accurate). Image layout transposed:
[cols -> partitions, rows -> free]; 98-col slabs; conv inputs are
DMA-built 128/102-col windows. Horizontal replicate-pad = replicated
edge-col tiles; vertical replicate-pad = maintained pad rows (masked so
only the true image top/bottom cores blend).

Sharding: 8 cores x (H/8) output rows, one launch per outer iteration
(one compiled program; d_i-dependent Toeplitz tables are inputs).
"""

import math
import numpy as np

import concourse.bacc as bacc
import concourse.mybir as mybir
import concourse.tile as tile
from concourse.bass_utils import run_bass_kernel_spmd

F32 = mybir.dt.float32
F32R = mybir.dt.float32r

N_ITER = 5
N_IN = 2
LAMBD = 0.005
BETA = (np.array([0.0, 1.0, 4.0, 16.0, 64.0, 256.0, 1024.0, 4096.0,
                  16384.0, 65536.0]) * 0.001 / 10.0 * 81.0)

KS, HKS = 31, 15
DS, HDS = 5, 2
S = HKS + HDS  # 17 halo rows consumed per inner step


def _toeplitz(rows, win_w, out_w, pad, R):
    A, T = rows.shape
    tabs = np.zeros((A, win_w, out_w), dtype=np.float32)
    for j in range(out_w):
        for t in range(T):
            c = j + pad + t - R
            if 0 <= c < win_w:
                tabs[:, c, j] = rows[:, t]
    return tabs


def _flat(tabs):
    A, P, O = tabs.shape
    return np.ascontiguousarray(tabs.transpose(1, 0, 2)).reshape(P, A * O)


def _g_offsets(g):
    nz = [a for a in range(DS) if np.any(g[a] != 0)]
    return [a - HDS for a in (nz or [HDS])]


def _cc_offsets(weight):
    offs = {0}
    for chn in (0, 1):
        for a in range(DS):
            if np.any(weight[chn, 0, a] != 0):
                for p in range(DS):
                    offs.add((p - HDS) + (a - HDS))
    return sorted(offs)


def make_tables(k2d, d_i, weight, offsG1, offsG2):
    tabK = _flat(_toeplitz(k2d, 128, 98, HKS, HKS))
    g1, g2 = weight[0, 0], weight[1, 0]
    tG1 = _flat(_toeplitz(np.stack([g1[v + HDS] for v in offsG1]),
                          128, 98, HKS, HDS))
    tG2 = _flat(_toeplitz(np.stack([g2[v + HDS] for v in offsG2]),
                          128, 98, HKS, HDS))
    tIx = _flat(_toeplitz(np.ones((1, 1), np.float32), 128, 98, HKS, 0))
    return dict(
        tabK=tabK, tabIx=tIx, tabG1=tG1, tabG2=tG2,
        tabD0y=_flat(_toeplitz(d_i[0], 102, 98, HDS, HDS)),
        tabD1=_flat(_toeplitz(d_i[1], 102, 98, HDS, HDS)),
        tabD2=_flat(_toeplitz(d_i[2], 102, 98, HDS, HDS)),
        tabD0=_flat(_toeplitz(-d_i[0], 102, 98, HDS, HDS)),
        tabD1n=_flat(_toeplitz(-d_i[1], 102, 98, HDS, HDS)),
        tabD2n=_flat(_toeplitz(-d_i[2], 102, 98, HDS, HDS)),
        idw=np.eye(98, dtype=np.float32))


class Builder:
    def __init__(self, W, OWN, n_ch, n_cores, offsG1, offsG2):
        self.W, self.OWN, self.n_ch, self.n_cores = W, OWN, n_ch, n_cores
        self.NB = math.ceil(W / 98)
        self.WPAD = self.NB * 98
        self.BAND = OWN + 4 * S
        self.L1 = OWN + 2 * S
        self.LW = self.L1
        self.LZ = self.LW + 2 * HDS
        self.offsG1, self.offsG2 = offsG1, offsG2
        self.fake = self.WPAD - W
        self.ERW = HKS + self.fake
        self.ERS = HDS + self.fake

    def build(self):
        W, NB, BAND, n_ch = self.W, self.NB, self.BAND, self.n_ch
        LW, LZ = self.LW, self.LZ
        offsG1, offsG2 = self.offsG1, self.offsG2
        nvG1, nvG2 = len(offsG1), len(offsG2)
        ERW, ERS = self.ERW, self.ERS
        zr0 = S - HDS
        lastb, lastp = divmod(W - 1, 98)

        nc = bacc.Bacc("TRN2", target_bir_lowering=False, debug=False,
                       num_devices=self.n_cores)
        din = lambda n, s, dt=F32R: nc.dram_tensor(
            n, s, dt, kind="ExternalInput").ap()
        x_d = din("x", (n_ch, self.WPAD, BAND))
        y_d = din("y", (n_ch, self.WPAD, BAND))
        tabK_d = din("tabK", (128, KS * 98))
        tabIx_d = din("tabIx", (128, 98))
        tabG1_d = din("tabG1", (128, nvG1 * 98))
        tabG2_d = din("tabG2", (128, nvG2 * 98))
        tabD0y_d = din("tabD0y", (102, DS * 98))
        tabD1_d = din("tabD1", (102, DS * 98))
        tabD2_d = din("tabD2", (102, DS * 98))
        tabD0_d = din("tabD0", (102, DS * 98))
        tabD1n_d = din("tabD1n", (102, DS * 98))
        tabD2n_d = din("tabD2n", (102, DS * 98))
        idw_d = din("idw", (98, 98))
        lam_d = din("lam", (98, 1), F32)
        mtop_d = din("mtop", (98, 1), F32)
        mbot_d = din("mbot", (98, 1), F32)
        mti_d = din("mti", (98, 2), mybir.dt.int32)
        mbi_d = din("mbi", (98, 2), mybir.dt.int32)
        out_d = nc.dram_tensor("o", (n_ch, self.WPAD, self.OWN), F32,
                               kind="ExternalOutput").ap()

        def pieces(c0, ww):
            res, c = [], c0
            while c < c0 + ww:
                if c < 0:
                    n = min(-c, c0 + ww - c)
                    res.append((c - c0, "L", 0, n))
                elif c >= W:
                    n = c0 + ww - c
                    res.append((c - c0, "R", 0, n))
                else:
                    b, p = divmod(c, 98)
                    n = min(98 - p, c0 + ww - c, W - c)
                    res.append((c - c0, "S", c, n))
                c += n
            return res

        xplan = [pieces(98 * B - HKS, 128) for B in range(NB)]
        splan = [pieces(98 * B - HDS, 102) for B in range(NB)]

        with tile.TileContext(nc) as tc:
            with tc.tile_pool(name="tabs", bufs=1) as tabp, \
                 tc.tile_pool(name="mast", bufs=1) as mast, \
                 tc.tile_pool(name="xw", bufs=NB + 2) as xwp, \
                 tc.tile_pool(name="sw", bufs=4) as swp, \
                 tc.tile_pool(name="zsl", bufs=4) as zslp, \
                 tc.tile_pool(name="edg", bufs=2) as edgp, \
                 tc.tile_pool(name="ps", bufs=4, space="PSUM") as pp, \
                 tc.tile_pool(name="ps2", bufs=4, space="PSUM") as pp2:

                _dmaeng = [nc.sync, nc.gpsimd, nc.scalar, nc.gpsimd]
                _dmaidx = [0]

                def dma(out, in_):
                    e = _dmaeng[_dmaidx[0] % 4]
                    _dmaidx[0] += 1
                    e.dma_start(out=out, in_=in_)

                def load_tab(d, p, w_, tag):
                    t = tabp.tile([p, w_], F32R, tag=tag)
                    dma(out=t[:, :], in_=d[:, :])
                    return t

                tK = load_tab(tabK_d, 128, KS * 98, "tK")
                tIx = load_tab(tabIx_d, 128, 98, "tIx")
                tG1 = load_tab(tabG1_d, 128, nvG1 * 98, "tG1")
                tG2 = load_tab(tabG2_d, 128, nvG2 * 98, "tG2")
                tD0y = load_tab(tabD0y_d, 102, DS * 98, "tD0y")
                tD1 = load_tab(tabD1_d, 102, DS * 98, "tD1")
                tD2 = load_tab(tabD2_d, 102, DS * 98, "tD2")
                tD0 = load_tab(tabD0_d, 102, DS * 98, "tD0")
                tD1n = load_tab(tabD1n_d, 102, DS * 98, "tD1n")
                tD2n = load_tab(tabD2n_d, 102, DS * 98, "tD2n")
                tIw = load_tab(idw_d, 98, 98, "tIw")
                lam = tabp.tile([98, 1], F32, tag="lam")
                dma(out=lam[:, :], in_=lam_d[:, :])
                mtop = tabp.tile([98, 1], F32, tag="mtop")
                dma(out=mtop[:, :], in_=mtop_d[:, :])
                mbot = tabp.tile([98, 1], F32, tag="mbot")
                dma(out=mbot[:, :], in_=mbot_d[:, :])
                mti = tabp.tile([98, 2], mybir.dt.int32, tag="mti")
                dma(out=mti[:, :], in_=mti_d[:, :])
                mbi = tabp.tile([98, 2], mybir.dt.int32, tag="mbi")
                dma(out=mbi[:, :], in_=mbi_d[:, :])

                def doubling(t, width):
                    k = 1
                    while k < width:
                        n = min(k, width - k)
                        dma(out=t[k:k + n, :], in_=t[0:n, :])
                        k += n

                def edge_from(ap_onecol, width, rows, tag):
                    e = edgp.tile([width, rows], F32R, tag=tag)
                    dma(out=e[0:1, :], in_=ap_onecol)
                    doubling(e, width)
                    return e

                I32 = mybir.dt.int32

                def blend2(out_ap, src_1col, imask):
                    nc.vector.copy_predicated(
                        out_ap.bitcast(I32), imask[:, 0:2],
                        src_1col.bitcast(I32).broadcast_to((98, 2)))

                def blend(out_ap, src_1col, mask, tmp_pool_tag, shape):
                    tmp = zslp.tile(list(shape), F32R, tag=tmp_pool_tag)
                    t_ap = tmp[tuple(slice(0, d_) for d_ in shape)]
                    nc.vector.tensor_sub(t_ap, src_1col.broadcast_to(shape),
                                         out_ap)
                    nc.vector.scalar_tensor_tensor(
                        out_ap, t_ap, mask[:, :1], out_ap,
                        mybir.AluOpType.mult, mybir.AluOpType.add)

                def softshrink(dst, src_ap, tmp):
                    nc.vector.tensor_scalar(dst[:, :], src_ap, lam[:, :1],
                                            0.0, mybir.AluOpType.subtract,
                                            mybir.AluOpType.max)
                    nc.vector.tensor_scalar(tmp[:, :], src_ap, lam[:, :1],
                                            0.0, mybir.AluOpType.add,
                                            mybir.AluOpType.min)
                    nc.vector.tensor_add(dst[:, :], dst[:, :], tmp[:, :])

                for ch in range(n_ch):
                    xs = mast.tile([98, NB, BAND], F32R, tag="xs")
                    for b in range(NB):
                        nc.sync.dma_start(
                            out=xs[:, b, :],
                            in_=x_d[ch, 98 * b:98 * b + 98, :])
                    ws = mast.tile([98, NB, LW], F32R, tag="ws")

                    # ---- x0 windows (serve z/w phase AND step 1) ----
                    eLx = edge_from(xs[0:1, 0, :], HKS, BAND, "xeL")
                    eRx = edge_from(xs[lastp:lastp + 1, lastb, :], ERW, BAND,
                                    "xeR")

                    def xwin(B, base, nrows, wb):
                        win = xwp.tile([128, nrows], F32R, tag="xw")
                        for dst, kind, off, n in xplan[B]:
                            if kind == "S":
                                b, p = divmod(off, 98)
                                nc.sync.dma_start(
                                    out=win[dst:dst + n, :],
                                    in_=xs[p:p + n, b,
                                           base:base + nrows])
                            elif kind == "L":
                                nc.sync.dma_start(
                                    out=win[dst:dst + n, :],
                                    in_=eLx[0:n, base:base + nrows]
                                     if wb is None else
                                    wb[0][0:n, base:base + nrows])
                            else:
                                nc.sync.dma_start(
                                    out=win[dst:dst + n, :],
                                    in_=eRx[0:n, base:base + nrows]
                                     if wb is None else
                                    wb[1][0:n, base:base + nrows])
                        return win

                    xw0 = {B: xwin(B, 0, BAND, None) for B in range(NB)}

                    def swin(plan_B, slab_ap, eL, eR, rows, tag):
                        win = swp.tile([102, rows], F32R, tag=tag)
                        for dst, kind, off, n in plan_B:
                            if kind == "S":
                                dma(out=win[dst:dst + n, :],
                                                  in_=slab_ap(off, n))
                            elif kind == "L":
                                dma(out=win[dst:dst + n, :],
                                                  in_=eL[0:n, :rows])
                            else:
                                dma(out=win[dst:dst + n, :],
                                                  in_=eR[0:n, :rows])
                        return win

                    # ======== z/w phase ========
                    z1s, z2s = {}, {}

                    def make_z(B):
                        psz = pp.tile([98, LZ], F32, tag="acc1")
                        for i, v in enumerate(offsG1):
                            nc.tensor.matmul(psz[:, :],
                                             tG1[:, 98 * i:98 * i + 98],
                                             xw0[B][:, zr0 + v:zr0 + v + LZ],
                                             start=(i == 0),
                                             stop=(i == nvG1 - 1))
                        z1 = zslp.tile([98, LZ], F32R, tag="z1")
                        zt = zslp.tile([98, LZ], F32R, tag="zt")
                        softshrink(z1, psz[:, :], zt)
                        psz2 = pp.tile([98, LZ], F32, tag="acc1")
                        for i, v in enumerate(offsG2):
                            nc.tensor.matmul(psz2[:, :],
                                             tG2[:, 98 * i:98 * i + 98],
                                             xw0[B][:, zr0 + v:zr0 + v + LZ],
                                             start=(i == 0),
                                             stop=(i == nvG2 - 1))
                        z2 = zslp.tile([98, LZ], F32R, tag="z2")
                        softshrink(z2, psz2[:, :], zt)
                        for zz in (z1, z2):
                            td = S
                            blend2(zz[:, td:td + 2], zz[:, td + 2:td + 3], mti)
                            bd = BAND - 2 * S - (S - 2)
                            blend2(zz[:, bd:bd + 2], zz[:, bd - 1:bd], mbi)
                        z1s[B], z2s[B] = z1, z2

                    yeL = edge_from(y_d[ch, 0:1, zr0:zr0 + LZ], HDS, LZ,
                                    "yeL")
                    yeR = edge_from(y_d[ch, W - 1:W, zr0:zr0 + LZ], ERS, LZ,
                                    "yeR")
                    make_z(0)
                    ez1L = ez2L = ez1R = ez2R = None
                    for B in range(NB):
                        if B + 1 < NB:
                            make_z(B + 1)
                        if ez1L is None:
                            ez1L = edge_from(z1s[0][0:1, :], HDS, LZ, "z1L")
                            ez2L = edge_from(z2s[0][0:1, :], HDS, LZ, "z2L")
                        if B == NB - 1 and ez1R is None:
                            ez1R = edge_from(z1s[lastb][lastp:lastp + 1, :],
                                             ERS, LZ, "z1R")
                            ez2R = edge_from(z2s[lastb][lastp:lastp + 1, :],
                                             ERS, LZ, "z2R")
                        psw = pp2.tile([98, LW], F32, tag="acc2")
                        yw = swin(splan[B],
                                  lambda off, n: y_d[
                                      ch, off:off + n,
                                      zr0:zr0 + LZ],
                                  yeL, yeR, LZ, "yw")
                        for a in range(DS):
                            nc.tensor.matmul(psw[:, :],
                                             tD0y[:, 98 * a:98 * a + 98],
                                             yw[:, a:a + LW],
                                             start=(a == 0), stop=False)
                        z1w = swin(splan[B],
                                   lambda off, n: z1s[off // 98][
                                       off % 98:off % 98 + n, :],
                                   ez1L, ez1R, LZ, "z1w")
                        for a in range(DS):
                            nc.tensor.matmul(psw[:, :],
                                             tD1[:, 98 * a:98 * a + 98],
                                             z1w[:, a:a + LW],
                                             start=False, stop=False)
                        z2w = swin(splan[B],
                                   lambda off, n: z2s[off // 98][
                                       off % 98:off % 98 + n, :],
                                   ez2L, ez2R, LZ, "z2w")
                        for a in range(DS):
                            nc.tensor.matmul(psw[:, :],
                                             tD2[:, 98 * a:98 * a + 98],
                                             z2w[:, a:a + LW],
                                             start=False, stop=(a == DS - 1))
                        nc.vector.tensor_copy(ws[:, B, :], psw[:, :])
                        z1s.pop(B - 1, None)
                        z2s.pop(B - 1, None)

                    # ======== inner steps ========
                    for step in range(N_IN):
                        r0 = S * (step + 1)
                        L = BAND - 2 * r0
                        if step == 0:
                            wbase, xws = 0, xw0
                        else:
                            wbase = r0 - S
                            eLx2 = edge_from(xs[0:1, 0, :], HKS, BAND, "xeL")
                            eRx2 = edge_from(xs[lastp:lastp + 1, lastb, :],
                                             ERW, BAND, "xeR")
                            xws = {B: xwin(B, wbase, L + 4 + 2 * HKS,
                                           (eLx2, eRx2)) for B in range(NB)}
                        kxs = {}
                        g1s, g2s = {}, {}
                        ekxL = ekxR = None
                        eg = {}

                        def make_g(B):
                            for (tG, nvG, offsG, dct, tag) in (
                                    (tG1, nvG1, offsG1, g1s, "g1"),
                                    (tG2, nvG2, offsG2, g2s, "g2")):
                                psg = pp.tile([98, L + 4], F32, tag="acc1")
                                for i, v in enumerate(offsG):
                                    o = r0 - 2 + v - wbase
                                    nc.tensor.matmul(
                                        psg[:, :], tG[:, 98 * i:98 * i + 98],
                                        xws[B][:, o:o + L + 4],
                                        start=(i == 0), stop=(i == nvG - 1))
                                g = zslp.tile([98, L + 4], F32R, tag=tag)
                                nc.vector.tensor_copy(g[:, :], psg[:, :])
                                td = 2 * S - r0
                                blend2(g[:, td:td + 2], g[:, td + 2:td + 3],
                                       mti)
                                bd = BAND - 2 * S - (r0 - 2)
                                blend2(g[:, bd:bd + 2], g[:, bd - 1:bd], mbi)
                                dct[B] = g

                        def make_kx(B):
                            psk = pp.tile([98, L + 4], F32, tag="acc1")
                            for a in range(KS):
                                o = a + r0 - wbase - HKS - HDS
                                nc.tensor.matmul(psk[:, :],
                                                 tK[:, 98 * a:98 * a + 98],
                                                 xws[B][:, o:o + L + 4],
                                                 start=(a == 0),
                                                 stop=(a == KS - 1))
                            kx = zslp.tile([98, L + 4], F32R, tag="kx")
                            nc.vector.tensor_copy(kx[:, :], psk[:, :])
                            # vertical replicate blend: global rows -2,-1 <-
                            # row 0 (core 0); rows H, H+1 <- H-1 (last core)
                            td = 2 * S - r0
                            blend2(kx[:, td:td + 2], kx[:, td + 2:td + 3],
                                   mti)
                            bd = BAND - 2 * S - (r0 - 2)
                            blend2(kx[:, bd:bd + 2], kx[:, bd - 1:bd], mbi)
                            kxs[B] = kx

                        make_kx(0)
                        make_g(0)
                        for B in range(NB):
                            if B + 1 < NB:
                                make_kx(B + 1)
                                make_g(B + 1)
                            if ekxL is None:
                                ekxL = edge_from(kxs[0][0:1, :], HDS, L + 4,
                                                 "kxL")
                                eg["g1L"] = edge_from(g1s[0][0:1, :], HDS,
                                                      L + 4, "g1L")
                                eg["g2L"] = edge_from(g2s[0][0:1, :], HDS,
                                                      L + 4, "g2L")
                            if B == NB - 1 and ekxR is None:
                                ekxR = edge_from(
                                    kxs[lastb][lastp:lastp + 1, :], ERS,
                                    L + 4, "kxR")
                                eg["g1R"] = edge_from(
                                    g1s[lastb][lastp:lastp + 1, :], ERS,
                                    L + 4, "g1R")
                                eg["g2R"] = edge_from(
                                    g2s[lastb][lastp:lastp + 1, :], ERS,
                                    L + 4, "g2R")
                            psx = pp2.tile([98, L], F32, tag="acc2")
                            nc.tensor.matmul(
                                psx[:, :], tIw[:, :],
                                ws[:, B, r0 - S:r0 - S + L],
                                start=True, stop=False)
                            nc.tensor.matmul(psx[:, :], tIx[:, :98],
                                             xws[B][:, r0 - wbase:
                                                    r0 - wbase + L],
                                             start=False, stop=False)
                            kw = swin(splan[B],
                                      lambda off, n: kxs[off // 98][
                                          off % 98:off % 98 + n, :],
                                      ekxL, ekxR, L + 4, "kw")
                            for a in range(DS):
                                nc.tensor.matmul(psx[:, :],
                                                 tD0[:, 98 * a:98 * a + 98],
                                                 kw[:, a:a + L],
                                                 start=False, stop=False)
                            g1w = swin(splan[B],
                                       lambda off, n: g1s[off // 98][
                                           off % 98:off % 98 + n, :],
                                       eg.get("g1L"), eg.get("g1R"), L + 4, "g1w")
                            for a in range(DS):
                                nc.tensor.matmul(psx[:, :],
                                                 tD1n[:, 98 * a:98 * a + 98],
                                                 g1w[:, a:a + L],
                                                 start=False, stop=False)
                            g2w = swin(splan[B],
                                       lambda off, n: g2s[off // 98][
                                           off % 98:off % 98 + n, :],
                                       eg.get("g2L"), eg.get("g2R"), L + 4, "g2w")
                            for a in range(DS):
                                nc.tensor.matmul(psx[:, :],
                                                 tD2n[:, 98 * a:98 * a + 98],
                                                 g2w[:, a:a + L],
                                                 start=False,
                                                 stop=(a == DS - 1))
                            if step == N_IN - 1:
                                nc.vector.tensor_scalar(
                                    xs[:, B, r0:r0 + L], psx[:, :], 0.0, 1.0,
                                    mybir.AluOpType.max, mybir.AluOpType.min)
                            else:
                                nc.vector.tensor_copy(xs[:, B, r0:r0 + L],
                                                      psx[:, :])
                            kxs.pop(B - 1, None)
                            g1s.pop(B - 1, None)
                            g2s.pop(B - 1, None)
                        if step < N_IN - 1:
                            blend(xs[:, :, S:2 * S],
                                  xs[:, :, 2 * S:2 * S + 1], mtop,
                                  "btmp3", (98, NB, S))
                            blend(xs[:, :, BAND - 2 * S:BAND - S],
                                  xs[:, :, BAND - 2 * S - 1:BAND - 2 * S],
                                  mbot, "btmp3", (98, NB, S))

                    for b in range(NB):
                        nc.sync.dma_start(
                            out=out_d[ch, 98 * b:98 * b + 98, :],
                            in_=xs[:, b, 2 * S:2 * S + self.OWN].bitcast(F32))

        nc.compile()
        return nc


LAST_EXEC_NS = None


def run_chqs(input_img, k, d, weight, n_cores=8, runner=None, trace=False):
    B0, C, H, W = input_img.shape
    OWN = H // n_cores
    k2d = np.asarray(k, np.float32)[0, 0]
    d = np.asarray(d, np.float32)
    weight = np.asarray(weight, np.float32)
    offsG1 = _g_offsets(weight[0, 0])
    offsG2 = _g_offsets(weight[1, 0])
    bld = Builder(W, OWN, C, n_cores, offsG1, offsG2)
    nc = bld.build()
    NB, WPAD = bld.NB, bld.WPAD

    img = np.asarray(input_img, np.float32)[0]

    def to_planes(a):
        t = np.transpose(a, (0, 2, 1))
        if WPAD > W:
            t = np.concatenate(
                [t, np.repeat(t[:, W - 1:W, :], WPAD - W, axis=1)], axis=1)
        return np.ascontiguousarray(t)

    y_pl = to_planes(img)

    def band_of(pl, c):
        idx = np.clip(np.arange(OWN * c - 2 * S, OWN * c + OWN + 2 * S),
                      0, H - 1)
        return np.ascontiguousarray(pl[:, :, idx])

    y_bands = [band_of(y_pl, c) for c in range(n_cores)]
    mt = [np.full((98, 1), 1.0 if c == 0 else 0.0, np.float32)
          for c in range(n_cores)]
    mb = [np.full((98, 1), 1.0 if c == n_cores - 1 else 0.0, np.float32)
          for c in range(n_cores)]
    mti = [np.full((98, 2), 1 if c == 0 else 0, np.int32)
           for c in range(n_cores)]
    mbi = [np.full((98, 2), 1 if c == n_cores - 1 else 0, np.int32)
           for c in range(n_cores)]

    x_pl = y_pl.copy()
    for it in range(N_ITER):
        lamv = LAMBD / max(1e-4, float(BETA[it]))
        tabs = make_tables(k2d, d[it], weight, offsG1, offsG2)
        in_maps = []
        for c in range(n_cores):
            m = dict(tabs)
            m["x"] = band_of(x_pl, c)
            m["y"] = y_bands[c]
            m["lam"] = np.full((98, 1), lamv, np.float32)
            m["mtop"] = mt[c]
            m["mbot"] = mb[c]
            m["mti"] = mti[c]
            m["mbi"] = mbi[c]
            in_maps.append(m)
        if runner is None:
            res = run_bass_kernel_spmd(nc, in_maps, list(range(n_cores)),
                                       trace=trace)
            outs = res.results
            if res.exec_time_ns:
                global LAST_EXEC_NS
                LAST_EXEC_NS = (LAST_EXEC_NS or 0) + res.exec_time_ns
        else:
            outs = runner(nc, in_maps)
        for c in range(n_cores):
            x_pl[:, :, OWN * c:OWN * c + OWN] = outs[c]["o"]
    return np.ascontiguousarray(
        np.transpose(x_pl[:, :W, :], (0, 2, 1)))[None].astype(np.float32)


def kernel(input, k, d, weight):
    return run_chqs(input, k, d, weight, n_cores=8)


# revision 17
# speedup vs baseline: 1.4128x; 1.0801x over previous
"""CHQS deconvolution kernel for Trainium2 (8 NeuronCores).

Reference computation: 5 outer iterations of
  z = softshrink(G x, lam_i); then 2x { r0 = y - Kx; r1 = z - Gx;
  x += D_i * pad([r0, r1], 2) }; x = clip(x, 0, 1)
with K a 31x31 blur (replicate pad 15), G 2-channel finite-diff (pad 2),
D_i [3,5,5] (pad 2).

Implementation identity (replicate-pad is linear: pad(a-b) = pad(a)-pad(b)):
  x_new = x + w - D0*pad(Kx) - CC*x
  w  = D0*pad(y) + D1*pad(z1) + D2*pad(z2)    (per outer iteration)
  CC = D1*G1 + D2*G2                          (composed 9x9 conv)
All convs run on the tensor engine as per-kernel-row Toeplitz matmuls in
float32r (full-rate, ~1e-4 accurate). Image layout transposed:
[cols -> partitions, rows -> free]; 98-col slabs; conv inputs are
DMA-built 128/102-col windows. Horizontal replicate-pad = replicated
edge-col tiles; vertical replicate-pad = maintained pad rows (masked so
only the true image top/bottom cores blend).

Sharding: 8 cores x (H/8) output rows, one launch per outer iteration
(one compiled program; d_i-dependent Toeplitz tables are inputs).
"""

import math
import numpy as np

import concourse.bacc as bacc
import concourse.mybir as mybir
import concourse.tile as tile
from concourse.bass_utils import run_bass_kernel_spmd

F32 = mybir.dt.float32
F32R = mybir.dt.float32r

N_ITER = 5
N_IN = 2
LAMBD = 0.005
BETA = (np.array([0.0, 1.0, 4.0, 16.0, 64.0, 256.0, 1024.0, 4096.0,
                  16384.0, 65536.0]) * 0.001 / 10.0 * 81.0)

KS, HKS = 31, 15
DS, HDS = 5, 2
S = HKS + HDS  # 17 halo rows consumed per inner step


def _toeplitz(rows, win_w, out_w, pad, R):
    A, T = rows.shape
    tabs = np.zeros((A, win_w, out_w), dtype=np.float32)
    for j in range(out_w):
        for t in range(T):
            c = j + pad + t - R
            if 0 <= c < win_w:
                tabs[:, c, j] = rows[:, t]
    return tabs


def _flat(tabs):
    A, P, O = tabs.shape
    return np.ascontiguousarray(tabs.transpose(1, 0, 2)).reshape(P, A * O)


def _g_offsets(g):
    nz = [a for a in range(DS) if np.any(g[a] != 0)]
    return [a - HDS for a in (nz or [HDS])]


def _cc_offsets(weight):
    offs = {0}
    for chn in (0, 1):
        for a in range(DS):
            if np.any(weight[chn, 0, a] != 0):
                for p in range(DS):
                    offs.add((p - HDS) + (a - HDS))
    return sorted(offs)


def make_tables(k2d, d_i, weight, offsG1, offsG2):
    tabK = _flat(_toeplitz(k2d, 128, 98, HKS, HKS))
    g1, g2 = weight[0, 0], weight[1, 0]
    tG1 = _flat(_toeplitz(np.stack([g1[v + HDS] for v in offsG1]),
                          128, 98, HKS, HDS))
    tG2 = _flat(_toeplitz(np.stack([g2[v + HDS] for v in offsG2]),
                          128, 98, HKS, HDS))
    tIx = _flat(_toeplitz(np.ones((1, 1), np.float32), 128, 98, HKS, 0))
    return dict(
        tabK=tabK, tabIx=tIx, tabG1=tG1, tabG2=tG2,
        tabD0y=_flat(_toeplitz(d_i[0], 102, 98, HDS, HDS)),
        tabD1=_flat(_toeplitz(d_i[1], 102, 98, HDS, HDS)),
        tabD2=_flat(_toeplitz(d_i[2], 102, 98, HDS, HDS)),
        tabD0=_flat(_toeplitz(-d_i[0], 102, 98, HDS, HDS)),
        tabD1n=_flat(_toeplitz(-d_i[1], 102, 98, HDS, HDS)),
        tabD2n=_flat(_toeplitz(-d_i[2], 102, 98, HDS, HDS)),
        idw=np.eye(98, dtype=np.float32))


class Builder:
    def __init__(self, W, OWN, n_ch, n_cores, offsG1, offsG2):
        self.W, self.OWN, self.n_ch, self.n_cores = W, OWN, n_ch, n_cores
        self.NB = math.ceil(W / 98)
        self.WPAD = self.NB * 98
        self.BAND = OWN + 4 * S
        self.L1 = OWN + 2 * S
        self.LW = self.L1
        self.LZ = self.LW + 2 * HDS
        self.offsG1, self.offsG2 = offsG1, offsG2
        self.fake = self.WPAD - W
        self.ERW = HKS + self.fake
        self.ERS = HDS + self.fake

    def build(self):
        W, NB, BAND, n_ch = self.W, self.NB, self.BAND, self.n_ch
        LW, LZ = self.LW, self.LZ
        offsG1, offsG2 = self.offsG1, self.offsG2
        nvG1, nvG2 = len(offsG1), len(offsG2)
        ERW, ERS = self.ERW, self.ERS
        zr0 = S - HDS
        lastb, lastp = divmod(W - 1, 98)

        nc = bacc.Bacc("TRN2", target_bir_lowering=False, debug=False,
                       num_devices=self.n_cores)
        din = lambda n, s, dt=F32R: nc.dram_tensor(
            n, s, dt, kind="ExternalInput").ap()
        x_d = din("x", (n_ch, self.WPAD, BAND))
        y_d = din("y", (n_ch, self.WPAD, BAND))
        tabK_d = din("tabK", (128, KS * 98))
        tabIx_d = din("tabIx", (128, 98))
        tabG1_d = din("tabG1", (128, nvG1 * 98))
        tabG2_d = din("tabG2", (128, nvG2 * 98))
        tabD0y_d = din("tabD0y", (102, DS * 98))
        tabD1_d = din("tabD1", (102, DS * 98))
        tabD2_d = din("tabD2", (102, DS * 98))
        tabD0_d = din("tabD0", (102, DS * 98))
        tabD1n_d = din("tabD1n", (102, DS * 98))
        tabD2n_d = din("tabD2n", (102, DS * 98))
        idw_d = din("idw", (98, 98))
        lam_d = din("lam", (98, 1), F32)
        mtop_d = din("mtop", (98, 1), F32)
        mbot_d = din("mbot", (98, 1), F32)
        mti_d = din("mti", (98, 2), mybir.dt.int32)
        mbi_d = din("mbi", (98, 2), mybir.dt.int32)
        out_d = nc.dram_tensor("o", (n_ch, self.WPAD, self.OWN), F32,
                               kind="ExternalOutput").ap()

        def pieces(c0, ww):
            res, c = [], c0
            while c < c0 + ww:
                if c < 0:
                    n = min(-c, c0 + ww - c)
                    res.append((c - c0, "L", 0, n))
                elif c >= W:
                    n = c0 + ww - c
                    res.append((c - c0, "R", 0, n))
                else:
                    b, p = divmod(c, 98)
                    n = min(98 - p, c0 + ww - c, W - c)
                    res.append((c - c0, "S", c, n))
                c += n
            return res

        xplan = [pieces(98 * B - HKS, 128) for B in range(NB)]
        splan = [pieces(98 * B - HDS, 102) for B in range(NB)]

        with tile.TileContext(nc) as tc:
            with tc.tile_pool(name="tabs", bufs=1) as tabp, \
                 tc.tile_pool(name="mast", bufs=1) as mast, \
                 tc.tile_pool(name="xsp", bufs=2) as xsp, \
                 tc.tile_pool(name="xw", bufs=NB + 2) as xwp, \
                 tc.tile_pool(name="sw", bufs=3) as swp, \
                 tc.tile_pool(name="zsl", bufs=4) as zslp, \
                 tc.tile_pool(name="edg", bufs=1) as edgp, \
                 tc.tile_pool(name="ps", bufs=4, space="PSUM") as pp, \
                 tc.tile_pool(name="ps2", bufs=4, space="PSUM") as pp2:

                _dmaeng = [nc.gpsimd, nc.scalar, nc.sync, nc.gpsimd, nc.scalar, nc.gpsimd]
                _dmaidx = [0]

                def dma(out, in_):
                    e = _dmaeng[_dmaidx[0] % 6]
                    _dmaidx[0] += 1
                    e.dma_start(out=out, in_=in_)

                def load_tab(d, p, w_, tag):
                    t = tabp.tile([p, w_], F32R, tag=tag)
                    dma(out=t[:, :], in_=d[:, :])
                    return t

                tK = load_tab(tabK_d, 128, KS * 98, "tK")
                tIx = load_tab(tabIx_d, 128, 98, "tIx")
                tG1 = load_tab(tabG1_d, 128, nvG1 * 98, "tG1")
                tG2 = load_tab(tabG2_d, 128, nvG2 * 98, "tG2")
                tD0y = load_tab(tabD0y_d, 102, DS * 98, "tD0y")
                tD1 = load_tab(tabD1_d, 102, DS * 98, "tD1")
                tD2 = load_tab(tabD2_d, 102, DS * 98, "tD2")
                tD0 = load_tab(tabD0_d, 102, DS * 98, "tD0")
                tD1n = load_tab(tabD1n_d, 102, DS * 98, "tD1n")
                tD2n = load_tab(tabD2n_d, 102, DS * 98, "tD2n")
                tIw = load_tab(idw_d, 98, 98, "tIw")
                lam = tabp.tile([98, 1], F32, tag="lam")
                dma(out=lam[:, :], in_=lam_d[:, :])
                mtop = tabp.tile([98, 1], F32, tag="mtop")
                dma(out=mtop[:, :], in_=mtop_d[:, :])
                mbot = tabp.tile([98, 1], F32, tag="mbot")
                dma(out=mbot[:, :], in_=mbot_d[:, :])
                mti = tabp.tile([98, 2], mybir.dt.int32, tag="mti")
                dma(out=mti[:, :], in_=mti_d[:, :])
                mbi = tabp.tile([98, 2], mybir.dt.int32, tag="mbi")
                dma(out=mbi[:, :], in_=mbi_d[:, :])

                def doubling(t, width):
                    k = 1
                    while k < width:
                        n = min(k, width - k)
                        dma(out=t[k:k + n, :], in_=t[0:n, :])
                        k += n

                def edge_from(ap_onecol, width, rows, tag):
                    e = edgp.tile([width, rows], F32R, tag=tag)
                    dma(out=e[0:1, :], in_=ap_onecol)
                    doubling(e, width)
                    return e

                I32 = mybir.dt.int32

                def blend2(out_ap, src_1col, imask):
                    nc.vector.copy_predicated(
                        out_ap.bitcast(I32), imask[:, 0:2],
                        src_1col.bitcast(I32).broadcast_to((98, 2)))

                def blend(out_ap, src_1col, mask, tmp_pool_tag, shape):
                    tmp = zslp.tile(list(shape), F32R, tag=tmp_pool_tag)
                    t_ap = tmp[tuple(slice(0, d_) for d_ in shape)]
                    nc.vector.tensor_sub(t_ap, src_1col.broadcast_to(shape),
                                         out_ap)
                    nc.vector.scalar_tensor_tensor(
                        out_ap, t_ap, mask[:, :1], out_ap,
                        mybir.AluOpType.mult, mybir.AluOpType.add)

                def softshrink(dst, src_ap, tmp):
                    nc.vector.tensor_scalar(dst[:, :], src_ap, lam[:, :1],
                                            0.0, mybir.AluOpType.subtract,
                                            mybir.AluOpType.max)
                    nc.vector.tensor_scalar(tmp[:, :], src_ap, lam[:, :1],
                                            0.0, mybir.AluOpType.add,
                                            mybir.AluOpType.min)
                    nc.vector.tensor_add(dst[:, :], dst[:, :], tmp[:, :])

                for ch in range(n_ch):
                    xs = xsp.tile([98, NB, BAND], F32R, tag="xs")
                    for b in range(NB):
                        nc.sync.dma_start(
                            out=xs[:, b, :],
                            in_=x_d[ch, 98 * b:98 * b + 98, :])
                    ws = mast.tile([98, NB, LW], F32R, tag="ws")

                    # ---- x0 windows (serve z/w phase AND step 1) ----
                    eLx = edge_from(xs[0:1, 0, :], HKS, BAND, "xeL")
                    eRx = edge_from(xs[lastp:lastp + 1, lastb, :], ERW, BAND,
                                    "xeR")

                    def xwin(B, base, nrows, wb):
                        win = xwp.tile([128, nrows], F32R, tag="xw")
                        for dst, kind, off, n in xplan[B]:
                            if kind == "S":
                                b, p = divmod(off, 98)
                                nc.sync.dma_start(
                                    out=win[dst:dst + n, :],
                                    in_=xs[p:p + n, b,
                                           base:base + nrows])
                            elif kind == "L":
                                nc.sync.dma_start(
                                    out=win[dst:dst + n, :],
                                    in_=eLx[0:n, base:base + nrows]
                                     if wb is None else
                                    wb[0][0:n, base:base + nrows])
                            else:
                                nc.sync.dma_start(
                                    out=win[dst:dst + n, :],
                                    in_=eRx[0:n, base:base + nrows]
                                     if wb is None else
                                    wb[1][0:n, base:base + nrows])
                        return win

                    xw0 = {B: xwin(B, 0, BAND, None) for B in range(NB)}

                    def swin(plan_B, slab_ap, eL, eR, rows, tag):
                        win = swp.tile([102, rows], F32R, tag=tag)
                        for dst, kind, off, n in plan_B:
                            if kind == "S":
                                dma(out=win[dst:dst + n, :],
                                                  in_=slab_ap(off, n))
                            elif kind == "L":
                                dma(out=win[dst:dst + n, :],
                                                  in_=eL[0:n, :rows])
                            else:
                                dma(out=win[dst:dst + n, :],
                                                  in_=eR[0:n, :rows])
                        return win

                    # ======== z/w phase ========
                    z1s, z2s = {}, {}

                    def make_z(B):
                        psz = pp.tile([98, LZ], F32, tag="acc1")
                        for i, v in enumerate(offsG1):
                            nc.tensor.matmul(psz[:, :],
                                             tG1[:, 98 * i:98 * i + 98],
                                             xw0[B][:, zr0 + v:zr0 + v + LZ],
                                             start=(i == 0),
                                             stop=(i == nvG1 - 1))
                        z1 = zslp.tile([98, LZ], F32R, tag="z1")
                        zt = zslp.tile([98, LZ], F32R, tag="zt")
                        softshrink(z1, psz[:, :], zt)
                        psz2 = pp.tile([98, LZ], F32, tag="acc1")
                        for i, v in enumerate(offsG2):
                            nc.tensor.matmul(psz2[:, :],
                                             tG2[:, 98 * i:98 * i + 98],
                                             xw0[B][:, zr0 + v:zr0 + v + LZ],
                                             start=(i == 0),
                                             stop=(i == nvG2 - 1))
                        z2 = zslp.tile([98, LZ], F32R, tag="z2")
                        softshrink(z2, psz2[:, :], zt)
                        for zz in (z1, z2):
                            td = S
                            blend2(zz[:, td:td + 2], zz[:, td + 2:td + 3], mti)
                            bd = BAND - 2 * S - (S - 2)
                            blend2(zz[:, bd:bd + 2], zz[:, bd - 1:bd], mbi)
                        z1s[B], z2s[B] = z1, z2

                    yeL = edge_from(y_d[ch, 0:1, zr0:zr0 + LZ], HDS, LZ,
                                    "yeL")
                    yeR = edge_from(y_d[ch, W - 1:W, zr0:zr0 + LZ], ERS, LZ,
                                    "yeR")
                    make_z(0)
                    ez1L = ez2L = ez1R = ez2R = None
                    for B in range(NB):
                        if B + 1 < NB:
                            make_z(B + 1)
                        if ez1L is None:
                            ez1L = edge_from(z1s[0][0:1, :], HDS, LZ, "z1L")
                            ez2L = edge_from(z2s[0][0:1, :], HDS, LZ, "z2L")
                        if B == NB - 1 and ez1R is None:
                            ez1R = edge_from(z1s[lastb][lastp:lastp + 1, :],
                                             ERS, LZ, "z1R")
                            ez2R = edge_from(z2s[lastb][lastp:lastp + 1, :],
                                             ERS, LZ, "z2R")
                        psw = pp2.tile([98, LW], F32, tag="acc2")
                        yw = swin(splan[B],
                                  lambda off, n: y_d[
                                      ch, off:off + n,
                                      zr0:zr0 + LZ],
                                  yeL, yeR, LZ, "yw")
                        for a in range(DS):
                            nc.tensor.matmul(psw[:, :],
                                             tD0y[:, 98 * a:98 * a + 98],
                                             yw[:, a:a + LW],
                                             start=(a == 0), stop=False)
                        z1w = swin(splan[B],
                                   lambda off, n: z1s[off // 98][
                                       off % 98:off % 98 + n, :],
                                   ez1L, ez1R, LZ, "z1w")
                        for a in range(DS):
                            nc.tensor.matmul(psw[:, :],
                                             tD1[:, 98 * a:98 * a + 98],
                                             z1w[:, a:a + LW],
                                             start=False, stop=False)
                        z2w = swin(splan[B],
                                   lambda off, n: z2s[off // 98][
                                       off % 98:off % 98 + n, :],
                                   ez2L, ez2R, LZ, "z2w")
                        for a in range(DS):
                            nc.tensor.matmul(psw[:, :],
                                             tD2[:, 98 * a:98 * a + 98],
                                             z2w[:, a:a + LW],
                                             start=False, stop=(a == DS - 1))
                        nc.vector.tensor_copy(ws[:, B, :], psw[:, :])
                        z1s.pop(B - 1, None)
                        z2s.pop(B - 1, None)

                    # ======== inner steps ========
                    for step in range(N_IN):
                        r0 = S * (step + 1)
                        L = BAND - 2 * r0
                        if step == 0:
                            wbase, xws = 0, xw0
                        else:
                            wbase = r0 - S
                            eLx2 = edge_from(xs[0:1, 0, :], HKS, BAND, "xeL")
                            eRx2 = edge_from(xs[lastp:lastp + 1, lastb, :],
                                             ERW, BAND, "xeR")
                            xws = {B: xwin(B, wbase, L + 4 + 2 * HKS,
                                           (eLx2, eRx2)) for B in range(NB)}
                        kxs = {}
                        g1s, g2s = {}, {}
                        ekxL = ekxR = None
                        eg = {}

                        def make_g(B):
                            for (tG, nvG, offsG, dct, tag) in (
                                    (tG1, nvG1, offsG1, g1s, "g1"),
                                    (tG2, nvG2, offsG2, g2s, "g2")):
                                psg = pp.tile([98, L + 4], F32, tag="acc1")
                                for i, v in enumerate(offsG):
                                    o = r0 - 2 + v - wbase
                                    nc.tensor.matmul(
                                        psg[:, :], tG[:, 98 * i:98 * i + 98],
                                        xws[B][:, o:o + L + 4],
                                        start=(i == 0), stop=(i == nvG - 1))
                                g = zslp.tile([98, L + 4], F32R, tag=tag)
                                nc.vector.tensor_copy(g[:, :], psg[:, :])
                                td = 2 * S - r0
                                blend2(g[:, td:td + 2], g[:, td + 2:td + 3],
                                       mti)
                                bd = BAND - 2 * S - (r0 - 2)
                                blend2(g[:, bd:bd + 2], g[:, bd - 1:bd], mbi)
                                dct[B] = g

                        def make_kx(B):
                            psk = pp.tile([98, L + 4], F32, tag="acc1")
                            for a in range(KS):
                                o = a + r0 - wbase - HKS - HDS
                                nc.tensor.matmul(psk[:, :],
                                                 tK[:, 98 * a:98 * a + 98],
                                                 xws[B][:, o:o + L + 4],
                                                 start=(a == 0),
                                                 stop=(a == KS - 1))
                            kx = zslp.tile([98, L + 4], F32R, tag="kx")
                            nc.vector.tensor_copy(kx[:, :], psk[:, :])
                            # vertical replicate blend: global rows -2,-1 <-
                            # row 0 (core 0); rows H, H+1 <- H-1 (last core)
                            td = 2 * S - r0
                            blend2(kx[:, td:td + 2], kx[:, td + 2:td + 3],
                                   mti)
                            bd = BAND - 2 * S - (r0 - 2)
                            blend2(kx[:, bd:bd + 2], kx[:, bd - 1:bd], mbi)
                            kxs[B] = kx

                        make_kx(0)
                        make_g(0)
                        for B in range(NB):
                            if B + 1 < NB:
                                make_kx(B + 1)
                                make_g(B + 1)
                            if ekxL is None:
                                ekxL = edge_from(kxs[0][0:1, :], HDS, L + 4,
                                                 "kxL")
                                eg["g1L"] = edge_from(g1s[0][0:1, :], HDS,
                                                      L + 4, "g1L")
                                eg["g2L"] = edge_from(g2s[0][0:1, :], HDS,
                                                      L + 4, "g2L")
                            if B == NB - 1 and ekxR is None:
                                ekxR = edge_from(
                                    kxs[lastb][lastp:lastp + 1, :], ERS,
                                    L + 4, "kxR")
                                eg["g1R"] = edge_from(
                                    g1s[lastb][lastp:lastp + 1, :], ERS,
                                    L + 4, "g1R")
                                eg["g2R"] = edge_from(
                                    g2s[lastb][lastp:lastp + 1, :], ERS,
                                    L + 4, "g2R")
                            psx = pp2.tile([98, L], F32, tag="acc2")
                            nc.tensor.matmul(
                                psx[:, :], tIw[:, :],
                                ws[:, B, r0 - S:r0 - S + L],
                                start=True, stop=False)
                            nc.tensor.matmul(psx[:, :], tIx[:, :98],
                                             xws[B][:, r0 - wbase:
                                                    r0 - wbase + L],
                                             start=False, stop=False)
                            kw = swin(splan[B],
                                      lambda off, n: kxs[off // 98][
                                          off % 98:off % 98 + n, :],
                                      ekxL, ekxR, L + 4, "kw")
                            for a in range(DS):
                                nc.tensor.matmul(psx[:, :],
                                                 tD0[:, 98 * a:98 * a + 98],
                                                 kw[:, a:a + L],
                                                 start=False, stop=False)
                            g1w = swin(splan[B],
                                       lambda off, n: g1s[off // 98][
                                           off % 98:off % 98 + n, :],
                                       eg.get("g1L"), eg.get("g1R"), L + 4, "g1w")
                            for a in range(DS):
                                nc.tensor.matmul(psx[:, :],
                                                 tD1n[:, 98 * a:98 * a + 98],
                                                 g1w[:, a:a + L],
                                                 start=False, stop=False)
                            g2w = swin(splan[B],
                                       lambda off, n: g2s[off // 98][
                                           off % 98:off % 98 + n, :],
                                       eg.get("g2L"), eg.get("g2R"), L + 4, "g2w")
                            for a in range(DS):
                                nc.tensor.matmul(psx[:, :],
                                                 tD2n[:, 98 * a:98 * a + 98],
                                                 g2w[:, a:a + L],
                                                 start=False,
                                                 stop=(a == DS - 1))
                            if step == N_IN - 1:
                                nc.vector.tensor_scalar(
                                    xs[:, B, r0:r0 + L], psx[:, :], 0.0, 1.0,
                                    mybir.AluOpType.max, mybir.AluOpType.min)
                            else:
                                nc.vector.tensor_copy(xs[:, B, r0:r0 + L],
                                                      psx[:, :])
                            kxs.pop(B - 1, None)
                            g1s.pop(B - 1, None)
                            g2s.pop(B - 1, None)
                        if step < N_IN - 1:
                            blend(xs[:, :, S:2 * S],
                                  xs[:, :, 2 * S:2 * S + 1], mtop,
                                  "btmp3", (98, NB, S))
                            blend(xs[:, :, BAND - 2 * S:BAND - S],
                                  xs[:, :, BAND - 2 * S - 1:BAND - 2 * S],
                                  mbot, "btmp3", (98, NB, S))

                    for b in range(NB):
                        nc.sync.dma_start(
                            out=out_d[ch, 98 * b:98 * b + 98, :],
                            in_=xs[:, b, 2 * S:2 * S + self.OWN].bitcast(F32))

        nc.compile()
        return nc


LAST_EXEC_NS = None


def run_chqs(input_img, k, d, weight, n_cores=8, runner=None, trace=False):
    B0, C, H, W = input_img.shape
    OWN = H // n_cores
    k2d = np.asarray(k, np.float32)[0, 0]
    d = np.asarray(d, np.float32)
    weight = np.asarray(weight, np.float32)
    offsG1 = _g_offsets(weight[0, 0])
    offsG2 = _g_offsets(weight[1, 0])
    bld = Builder(W, OWN, C, n_cores, offsG1, offsG2)
    nc = bld.build()
    NB, WPAD = bld.NB, bld.WPAD

    img = np.asarray(input_img, np.float32)[0]

    def to_planes(a):
        t = np.transpose(a, (0, 2, 1))
        if WPAD > W:
            t = np.concatenate(
                [t, np.repeat(t[:, W - 1:W, :], WPAD - W, axis=1)], axis=1)
        return np.ascontiguousarray(t)

    y_pl = to_planes(img)

    def band_of(pl, c):
        idx = np.clip(np.arange(OWN * c - 2 * S, OWN * c + OWN + 2 * S),
                      0, H - 1)
        return np.ascontiguousarray(pl[:, :, idx])

    y_bands = [band_of(y_pl, c) for c in range(n_cores)]
    mt = [np.full((98, 1), 1.0 if c == 0 else 0.0, np.float32)
          for c in range(n_cores)]
    mb = [np.full((98, 1), 1.0 if c == n_cores - 1 else 0.0, np.float32)
          for c in range(n_cores)]
    mti = [np.full((98, 2), 1 if c == 0 else 0, np.int32)
           for c in range(n_cores)]
    mbi = [np.full((98, 2), 1 if c == n_cores - 1 else 0, np.int32)
           for c in range(n_cores)]

    x_pl = y_pl.copy()
    for it in range(N_ITER):
        lamv = LAMBD / max(1e-4, float(BETA[it]))
        tabs = make_tables(k2d, d[it], weight, offsG1, offsG2)
        in_maps = []
        for c in range(n_cores):
            m = dict(tabs)
            m["x"] = band_of(x_pl, c)
            m["y"] = y_bands[c]
            m["lam"] = np.full((98, 1), lamv, np.float32)
            m["mtop"] = mt[c]
            m["mbot"] = mb[c]
            m["mti"] = mti[c]
            m["mbi"] = mbi[c]
            in_maps.append(m)
        if runner is None:
            res = run_bass_kernel_spmd(nc, in_maps, list(range(n_cores)),
                                       trace=trace)
            outs = res.results
            if res.exec_time_ns:
                global LAST_EXEC_NS
                LAST_EXEC_NS = (LAST_EXEC_NS or 0) + res.exec_time_ns
        else:
            outs = runner(nc, in_maps)
        for c in range(n_cores):
            x_pl[:, :, OWN * c:OWN * c + OWN] = outs[c]["o"]
    return np.ascontiguousarray(
        np.transpose(x_pl[:, :W, :], (0, 2, 1)))[None].astype(np.float32)


def kernel(input, k, d, weight):
    return run_chqs(input, k, d, weight, n_cores=8)


# revision 19
# speedup vs baseline: 1.4268x; 1.0099x over previous
"""CHQS deconvolution kernel for Trainium2 (8 NeuronCores).

Reference computation: 5 outer iterations of
  z = softshrink(G x, lam_i); then 2x { r0 = y - Kx; r1 = z - Gx;
  x += D_i * pad([r0, r1], 2) }; x = clip(x, 0, 1)
with K a 31x31 blur (replicate pad 15), G 2-channel finite-diff (pad 2),
D_i [3,5,5] (pad 2).

Implementation identity (replicate-pad is linear: pad(a-b) = pad(a)-pad(b)):
  x_new = x + w - D0*pad(Kx) - CC*x
  w  = D0*pad(y) + D1*pad(z1) + D2*pad(z2)    (per outer iteration)
  CC = D1*G1 + D2*G2                          (composed 9x9 conv)
All convs run on the tensor engine as per-kernel-row Toeplitz matmuls in
float32r (full-rate, ~1e-4 accurate). Image layout transposed:
[cols -> partitions, rows -> free]; 98-col slabs; conv inputs are
DMA-built 128/102-col windows. Horizontal replicate-pad = replicated
edge-col tiles; vertical replicate-pad = maintained pad rows (masked so
only the true image top/bottom cores blend).

Sharding: 8 cores x (H/8) output rows, one launch per outer iteration
(one compiled program; d_i-dependent Toeplitz tables are inputs).
"""

import math
import numpy as np

import concourse.bacc as bacc
import concourse.mybir as mybir
import concourse.tile as tile
from concourse.bass_utils import run_bass_kernel_spmd

F32 = mybir.dt.float32
F32R = mybir.dt.float32r

N_ITER = 5
N_IN = 2
LAMBD = 0.005
BETA = (np.array([0.0, 1.0, 4.0, 16.0, 64.0, 256.0, 1024.0, 4096.0,
                  16384.0, 65536.0]) * 0.001 / 10.0 * 81.0)

KS, HKS = 31, 15
DS, HDS = 5, 2
S = HKS + HDS  # 17 halo rows consumed per inner step


def _toeplitz(rows, win_w, out_w, pad, R):
    A, T = rows.shape
    tabs = np.zeros((A, win_w, out_w), dtype=np.float32)
    for j in range(out_w):
        for t in range(T):
            c = j + pad + t - R
            if 0 <= c < win_w:
                tabs[:, c, j] = rows[:, t]
    return tabs


def _flat(tabs):
    A, P, O = tabs.shape
    return np.ascontiguousarray(tabs.transpose(1, 0, 2)).reshape(P, A * O)


def _g_offsets(g):
    nz = [a for a in range(DS) if np.any(g[a] != 0)]
    return [a - HDS for a in (nz or [HDS])]


def _cc_offsets(weight):
    offs = {0}
    for chn in (0, 1):
        for a in range(DS):
            if np.any(weight[chn, 0, a] != 0):
                for p in range(DS):
                    offs.add((p - HDS) + (a - HDS))
    return sorted(offs)


def make_tables(k2d, d_i, weight, offsG1, offsG2):
    tabK = _flat(_toeplitz(k2d, 128, 98, HKS, HKS))
    g1, g2 = weight[0, 0], weight[1, 0]
    tG1 = _flat(_toeplitz(np.stack([g1[v + HDS] for v in offsG1]),
                          128, 98, HKS, HDS))
    tG2 = _flat(_toeplitz(np.stack([g2[v + HDS] for v in offsG2]),
                          128, 98, HKS, HDS))
    tIx = _flat(_toeplitz(np.ones((1, 1), np.float32), 128, 98, HKS, 0))
    return dict(
        tabK=tabK, tabIx=tIx, tabG1=tG1, tabG2=tG2,
        tabD0y=_flat(_toeplitz(d_i[0], 102, 98, HDS, HDS)),
        tabD1=_flat(_toeplitz(d_i[1], 102, 98, HDS, HDS)),
        tabD2=_flat(_toeplitz(d_i[2], 102, 98, HDS, HDS)),
        tabD0=_flat(_toeplitz(-d_i[0], 102, 98, HDS, HDS)),
        tabD1n=_flat(_toeplitz(-d_i[1], 102, 98, HDS, HDS)),
        tabD2n=_flat(_toeplitz(-d_i[2], 102, 98, HDS, HDS)),
        idw=np.eye(98, dtype=np.float32))


class Builder:
    def __init__(self, W, OWN, n_ch, n_cores, offsG1, offsG2):
        self.W, self.OWN, self.n_ch, self.n_cores = W, OWN, n_ch, n_cores
        self.NB = math.ceil(W / 98)
        self.WPAD = self.NB * 98
        self.BAND = OWN + 4 * S
        self.L1 = OWN + 2 * S
        self.LW = self.L1
        self.LZ = self.LW + 2 * HDS
        self.offsG1, self.offsG2 = offsG1, offsG2
        self.fake = self.WPAD - W
        self.ERW = HKS + self.fake
        self.ERS = HDS + self.fake

    def build(self):
        W, NB, BAND, n_ch = self.W, self.NB, self.BAND, self.n_ch
        LW, LZ = self.LW, self.LZ
        offsG1, offsG2 = self.offsG1, self.offsG2
        nvG1, nvG2 = len(offsG1), len(offsG2)
        ERW, ERS = self.ERW, self.ERS
        zr0 = S - HDS
        lastb, lastp = divmod(W - 1, 98)

        nc = bacc.Bacc("TRN2", target_bir_lowering=False, debug=False,
                       num_devices=self.n_cores)
        din = lambda n, s, dt=F32R: nc.dram_tensor(
            n, s, dt, kind="ExternalInput").ap()
        x_d = din("x", (n_ch, self.WPAD, BAND))
        y_d = din("y", (n_ch, self.WPAD, BAND))
        tabK_d = din("tabK", (128, KS * 98))
        tabIx_d = din("tabIx", (128, 98))
        tabG1_d = din("tabG1", (128, nvG1 * 98))
        tabG2_d = din("tabG2", (128, nvG2 * 98))
        tabD0y_d = din("tabD0y", (102, DS * 98))
        tabD1_d = din("tabD1", (102, DS * 98))
        tabD2_d = din("tabD2", (102, DS * 98))
        tabD0_d = din("tabD0", (102, DS * 98))
        tabD1n_d = din("tabD1n", (102, DS * 98))
        tabD2n_d = din("tabD2n", (102, DS * 98))
        idw_d = din("idw", (98, 98))
        lam_d = din("lam", (98, 1), F32)
        mtop_d = din("mtop", (98, 1), F32)
        mbot_d = din("mbot", (98, 1), F32)
        mti_d = din("mti", (98, 2), mybir.dt.int32)
        mbi_d = din("mbi", (98, 2), mybir.dt.int32)
        out_d = nc.dram_tensor("o", (n_ch, self.WPAD, self.OWN), F32,
                               kind="ExternalOutput").ap()

        def pieces(c0, ww):
            res, c = [], c0
            while c < c0 + ww:
                if c < 0:
                    n = min(-c, c0 + ww - c)
                    res.append((c - c0, "L", 0, n))
                elif c >= W:
                    n = c0 + ww - c
                    res.append((c - c0, "R", 0, n))
                else:
                    b, p = divmod(c, 98)
                    n = min(98 - p, c0 + ww - c, W - c)
                    res.append((c - c0, "S", c, n))
                c += n
            return res

        xplan = [pieces(98 * B - HKS, 128) for B in range(NB)]
        splan = [pieces(98 * B - HDS, 102) for B in range(NB)]

        with tile.TileContext(nc) as tc:
            with tc.tile_pool(name="tabs", bufs=1) as tabp, \
                 tc.tile_pool(name="mast", bufs=1) as mast, \
                 tc.tile_pool(name="xsp", bufs=2) as xsp, \
                 tc.tile_pool(name="xw", bufs=NB + 2) as xwp, \
                 tc.tile_pool(name="sw", bufs=3) as swp, \
                 tc.tile_pool(name="zsl", bufs=4) as zslp, \
                 tc.tile_pool(name="edg", bufs=1) as edgp, \
                 tc.tile_pool(name="ps", bufs=4, space="PSUM") as pp, \
                 tc.tile_pool(name="ps2", bufs=4, space="PSUM") as pp2:

                _dmaeng = [nc.gpsimd, nc.scalar, nc.sync, nc.gpsimd, nc.scalar, nc.gpsimd]
                _dmaidx = [0]

                def dma(out, in_):
                    e = _dmaeng[_dmaidx[0] % 6]
                    _dmaidx[0] += 1
                    e.dma_start(out=out, in_=in_)

                def load_tab(d, p, w_, tag):
                    t = tabp.tile([p, w_], F32R, tag=tag)
                    dma(out=t[:, :], in_=d[:, :])
                    return t

                tK = load_tab(tabK_d, 128, KS * 98, "tK")
                tIx = load_tab(tabIx_d, 128, 98, "tIx")
                tG1 = load_tab(tabG1_d, 128, nvG1 * 98, "tG1")
                tG2 = load_tab(tabG2_d, 128, nvG2 * 98, "tG2")
                tD0y = load_tab(tabD0y_d, 102, DS * 98, "tD0y")
                tD1 = load_tab(tabD1_d, 102, DS * 98, "tD1")
                tD2 = load_tab(tabD2_d, 102, DS * 98, "tD2")
                tD0 = load_tab(tabD0_d, 102, DS * 98, "tD0")
                tD1n = load_tab(tabD1n_d, 102, DS * 98, "tD1n")
                tD2n = load_tab(tabD2n_d, 102, DS * 98, "tD2n")
                tIw = load_tab(idw_d, 98, 98, "tIw")
                lam = tabp.tile([98, 1], F32, tag="lam")
                dma(out=lam[:, :], in_=lam_d[:, :])
                mtop = tabp.tile([98, 1], F32, tag="mtop")
                dma(out=mtop[:, :], in_=mtop_d[:, :])
                mbot = tabp.tile([98, 1], F32, tag="mbot")
                dma(out=mbot[:, :], in_=mbot_d[:, :])
                mti = tabp.tile([98, 2], mybir.dt.int32, tag="mti")
                dma(out=mti[:, :], in_=mti_d[:, :])
                mbi = tabp.tile([98, 2], mybir.dt.int32, tag="mbi")
                dma(out=mbi[:, :], in_=mbi_d[:, :])

                def doubling(t, width):
                    k = 1
                    while k < width:
                        n = min(k, width - k)
                        dma(out=t[k:k + n, :], in_=t[0:n, :])
                        k += n

                def edge_from(ap_onecol, width, rows, tag):
                    e = edgp.tile([width, rows], F32R, tag=tag)
                    dma(out=e[0:1, :], in_=ap_onecol)
                    doubling(e, width)
                    return e

                I32 = mybir.dt.int32

                def blend2(out_ap, src_1col, imask):
                    nc.vector.copy_predicated(
                        out_ap.bitcast(I32), imask[:, 0:2],
                        src_1col.bitcast(I32).broadcast_to((98, 2)))

                def blend(out_ap, src_1col, mask, tmp_pool_tag, shape):
                    tmp = zslp.tile(list(shape), F32R, tag=tmp_pool_tag)
                    t_ap = tmp[tuple(slice(0, d_) for d_ in shape)]
                    nc.vector.tensor_sub(t_ap, src_1col.broadcast_to(shape),
                                         out_ap)
                    nc.vector.scalar_tensor_tensor(
                        out_ap, t_ap, mask[:, :1], out_ap,
                        mybir.AluOpType.mult, mybir.AluOpType.add)

                def softshrink(dst, src_ap, tmp):
                    nc.vector.tensor_scalar(dst[:, :], src_ap, lam[:, :1],
                                            0.0, mybir.AluOpType.subtract,
                                            mybir.AluOpType.max)
                    nc.vector.tensor_scalar(tmp[:, :], src_ap, lam[:, :1],
                                            0.0, mybir.AluOpType.add,
                                            mybir.AluOpType.min)
                    nc.vector.tensor_add(dst[:, :], dst[:, :], tmp[:, :])

                for ch in range(n_ch):
                    xs = xsp.tile([98, NB, BAND], F32R, tag="xs")
                    for b in range(NB):
                        dma(out=xs[:, b, :],
                            in_=x_d[ch, 98 * b:98 * b + 98, :])
                    ws = mast.tile([98, NB, LW], F32R, tag="ws")

                    # ---- x0 windows (serve z/w phase AND step 1) ----
                    eLx = edge_from(xs[0:1, 0, :], HKS, BAND, "xeL")
                    eRx = edge_from(xs[lastp:lastp + 1, lastb, :], ERW, BAND,
                                    "xeR")

                    def xwin(B, base, nrows, wb):
                        win = xwp.tile([128, nrows], F32R, tag="xw")
                        for dst, kind, off, n in xplan[B]:
                            if kind == "S":
                                b, p = divmod(off, 98)
                                nc.sync.dma_start(
                                    out=win[dst:dst + n, :],
                                    in_=xs[p:p + n, b,
                                           base:base + nrows])
                            elif kind == "L":
                                nc.sync.dma_start(
                                    out=win[dst:dst + n, :],
                                    in_=eLx[0:n, base:base + nrows]
                                     if wb is None else
                                    wb[0][0:n, base:base + nrows])
                            else:
                                nc.sync.dma_start(
                                    out=win[dst:dst + n, :],
                                    in_=eRx[0:n, base:base + nrows]
                                     if wb is None else
                                    wb[1][0:n, base:base + nrows])
                        return win

                    xw0 = {B: xwin(B, 0, BAND, None) for B in range(NB)}

                    def swin(plan_B, slab_ap, eL, eR, rows, tag):
                        win = swp.tile([102, rows], F32R, tag=tag)
                        for dst, kind, off, n in plan_B:
                            if kind == "S":
                                dma(out=win[dst:dst + n, :],
                                                  in_=slab_ap(off, n))
                            elif kind == "L":
                                dma(out=win[dst:dst + n, :],
                                                  in_=eL[0:n, :rows])
                            else:
                                dma(out=win[dst:dst + n, :],
                                                  in_=eR[0:n, :rows])
                        return win

                    # ======== z/w phase ========
                    z1s, z2s = {}, {}

                    def make_z(B):
                        psz = pp.tile([98, LZ], F32, tag="acc1")
                        for i, v in enumerate(offsG1):
                            nc.tensor.matmul(psz[:, :],
                                             tG1[:, 98 * i:98 * i + 98],
                                             xw0[B][:, zr0 + v:zr0 + v + LZ],
                                             start=(i == 0),
                                             stop=(i == nvG1 - 1))
                        z1 = zslp.tile([98, LZ], F32R, tag="z1")
                        zt = zslp.tile([98, LZ], F32R, tag="zt")
                        softshrink(z1, psz[:, :], zt)
                        psz2 = pp.tile([98, LZ], F32, tag="acc1")
                        for i, v in enumerate(offsG2):
                            nc.tensor.matmul(psz2[:, :],
                                             tG2[:, 98 * i:98 * i + 98],
                                             xw0[B][:, zr0 + v:zr0 + v + LZ],
                                             start=(i == 0),
                                             stop=(i == nvG2 - 1))
                        z2 = zslp.tile([98, LZ], F32R, tag="z2")
                        softshrink(z2, psz2[:, :], zt)
                        for zz in (z1, z2):
                            td = S
                            blend2(zz[:, td:td + 2], zz[:, td + 2:td + 3], mti)
                            bd = BAND - 2 * S - (S - 2)
                            blend2(zz[:, bd:bd + 2], zz[:, bd - 1:bd], mbi)
                        z1s[B], z2s[B] = z1, z2

                    yeL = edge_from(y_d[ch, 0:1, zr0:zr0 + LZ], HDS, LZ,
                                    "yeL")
                    yeR = edge_from(y_d[ch, W - 1:W, zr0:zr0 + LZ], ERS, LZ,
                                    "yeR")
                    make_z(0)
                    ez1L = ez2L = ez1R = ez2R = None
                    for B in range(NB):
                        if B + 1 < NB:
                            make_z(B + 1)
                        if ez1L is None:
                            ez1L = edge_from(z1s[0][0:1, :], HDS, LZ, "z1L")
                            ez2L = edge_from(z2s[0][0:1, :], HDS, LZ, "z2L")
                        if B == NB - 1 and ez1R is None:
                            ez1R = edge_from(z1s[lastb][lastp:lastp + 1, :],
                                             ERS, LZ, "z1R")
                            ez2R = edge_from(z2s[lastb][lastp:lastp + 1, :],
                                             ERS, LZ, "z2R")
                        psw = pp2.tile([98, LW], F32, tag="acc2")
                        yw = swin(splan[B],
                                  lambda off, n: y_d[
                                      ch, off:off + n,
                                      zr0:zr0 + LZ],
                                  yeL, yeR, LZ, "yw")
                        for a in range(DS):
                            nc.tensor.matmul(psw[:, :],
                                             tD0y[:, 98 * a:98 * a + 98],
                                             yw[:, a:a + LW],
                                             start=(a == 0), stop=False)
                        z1w = swin(splan[B],
                                   lambda off, n: z1s[off // 98][
                                       off % 98:off % 98 + n, :],
                                   ez1L, ez1R, LZ, "z1w")
                        for a in range(DS):
                            nc.tensor.matmul(psw[:, :],
                                             tD1[:, 98 * a:98 * a + 98],
                                             z1w[:, a:a + LW],
                                             start=False, stop=False)
                        z2w = swin(splan[B],
                                   lambda off, n: z2s[off // 98][
                                       off % 98:off % 98 + n, :],
                                   ez2L, ez2R, LZ, "z2w")
                        for a in range(DS):
                            nc.tensor.matmul(psw[:, :],
                                             tD2[:, 98 * a:98 * a + 98],
                                             z2w[:, a:a + LW],
                                             start=False, stop=(a == DS - 1))
                        nc.vector.tensor_copy(ws[:, B, :], psw[:, :])
                        z1s.pop(B - 1, None)
                        z2s.pop(B - 1, None)

                    # ======== inner steps ========
                    for step in range(N_IN):
                        r0 = S * (step + 1)
                        L = BAND - 2 * r0
                        if step == 0:
                            wbase, xws = 0, xw0
                        else:
                            wbase = r0 - S
                            eLx2 = edge_from(xs[0:1, 0, :], HKS, BAND, "xeL")
                            eRx2 = edge_from(xs[lastp:lastp + 1, lastb, :],
                                             ERW, BAND, "xeR")
                            xws = {B: xwin(B, wbase, L + 4 + 2 * HKS,
                                           (eLx2, eRx2)) for B in range(NB)}
                        kxs = {}
                        g1s, g2s = {}, {}
                        ekxL = ekxR = None
                        eg = {}

                        def make_g(B):
                            for (tG, nvG, offsG, dct, tag) in (
                                    (tG1, nvG1, offsG1, g1s, "g1"),
                                    (tG2, nvG2, offsG2, g2s, "g2")):
                                psg = pp.tile([98, L + 4], F32, tag="acc1")
                                for i, v in enumerate(offsG):
                                    o = r0 - 2 + v - wbase
                                    nc.tensor.matmul(
                                        psg[:, :], tG[:, 98 * i:98 * i + 98],
                                        xws[B][:, o:o + L + 4],
                                        start=(i == 0), stop=(i == nvG - 1))
                                g = zslp.tile([98, L + 4], F32R, tag=tag)
                                nc.vector.tensor_copy(g[:, :], psg[:, :])
                                td = 2 * S - r0
                                blend2(g[:, td:td + 2], g[:, td + 2:td + 3],
                                       mti)
                                bd = BAND - 2 * S - (r0 - 2)
                                blend2(g[:, bd:bd + 2], g[:, bd - 1:bd], mbi)
                                dct[B] = g

                        def make_kx(B):
                            psk = pp.tile([98, L + 4], F32, tag="acc1")
                            for a in range(KS):
                                o = a + r0 - wbase - HKS - HDS
                                nc.tensor.matmul(psk[:, :],
                                                 tK[:, 98 * a:98 * a + 98],
                                                 xws[B][:, o:o + L + 4],
                                                 start=(a == 0),
                                                 stop=(a == KS - 1))
                            kx = zslp.tile([98, L + 4], F32R, tag="kx")
                            nc.vector.tensor_copy(kx[:, :], psk[:, :])
                            # vertical replicate blend: global rows -2,-1 <-
                            # row 0 (core 0); rows H, H+1 <- H-1 (last core)
                            td = 2 * S - r0
                            blend2(kx[:, td:td + 2], kx[:, td + 2:td + 3],
                                   mti)
                            bd = BAND - 2 * S - (r0 - 2)
                            blend2(kx[:, bd:bd + 2], kx[:, bd - 1:bd], mbi)
                            kxs[B] = kx

                        make_kx(0)
                        make_g(0)
                        for B in range(NB):
                            if B + 1 < NB:
                                make_kx(B + 1)
                                make_g(B + 1)
                            if ekxL is None:
                                ekxL = edge_from(kxs[0][0:1, :], HDS, L + 4,
                                                 "kxL")
                                eg["g1L"] = edge_from(g1s[0][0:1, :], HDS,
                                                      L + 4, "g1L")
                                eg["g2L"] = edge_from(g2s[0][0:1, :], HDS,
                                                      L + 4, "g2L")
                            if B == NB - 1 and ekxR is None:
                                ekxR = edge_from(
                                    kxs[lastb][lastp:lastp + 1, :], ERS,
                                    L + 4, "kxR")
                                eg["g1R"] = edge_from(
                                    g1s[lastb][lastp:lastp + 1, :], ERS,
                                    L + 4, "g1R")
                                eg["g2R"] = edge_from(
                                    g2s[lastb][lastp:lastp + 1, :], ERS,
                                    L + 4, "g2R")
                            psx = pp2.tile([98, L], F32, tag="acc2")
                            nc.tensor.matmul(
                                psx[:, :], tIw[:, :],
                                ws[:, B, r0 - S:r0 - S + L],
                                start=True, stop=False)
                            nc.tensor.matmul(psx[:, :], tIx[:, :98],
                                             xws[B][:, r0 - wbase:
                                                    r0 - wbase + L],
                                             start=False, stop=False)
                            kw = swin(splan[B],
                                      lambda off, n: kxs[off // 98][
                                          off % 98:off % 98 + n, :],
                                      ekxL, ekxR, L + 4, "kw")
                            for a in range(DS):
                                nc.tensor.matmul(psx[:, :],
                                                 tD0[:, 98 * a:98 * a + 98],
                                                 kw[:, a:a + L],
                                                 start=False, stop=False)
                            g1w = swin(splan[B],
                                       lambda off, n: g1s[off // 98][
                                           off % 98:off % 98 + n, :],
                                       eg.get("g1L"), eg.get("g1R"), L + 4, "g1w")
                            for a in range(DS):
                                nc.tensor.matmul(psx[:, :],
                                                 tD1n[:, 98 * a:98 * a + 98],
                                                 g1w[:, a:a + L],
                                                 start=False, stop=False)
                            g2w = swin(splan[B],
                                       lambda off, n: g2s[off // 98][
                                           off % 98:off % 98 + n, :],
                                       eg.get("g2L"), eg.get("g2R"), L + 4, "g2w")
                            for a in range(DS):
                                nc.tensor.matmul(psx[:, :],
                                                 tD2n[:, 98 * a:98 * a + 98],
                                                 g2w[:, a:a + L],
                                                 start=False,
                                                 stop=(a == DS - 1))
                            if step == N_IN - 1:
                                nc.vector.tensor_scalar(
                                    xs[:, B, r0:r0 + L], psx[:, :], 0.0, 1.0,
                                    mybir.AluOpType.max, mybir.AluOpType.min)
                            else:
                                nc.vector.tensor_copy(xs[:, B, r0:r0 + L],
                                                      psx[:, :])
                            kxs.pop(B - 1, None)
                            g1s.pop(B - 1, None)
                            g2s.pop(B - 1, None)
                        if step < N_IN - 1:
                            blend(xs[:, :, S:2 * S],
                                  xs[:, :, 2 * S:2 * S + 1], mtop,
                                  "btmp3", (98, NB, S))
                            blend(xs[:, :, BAND - 2 * S:BAND - S],
                                  xs[:, :, BAND - 2 * S - 1:BAND - 2 * S],
                                  mbot, "btmp3", (98, NB, S))

                    for b in range(NB):
                        nc.sync.dma_start(
                            out=out_d[ch, 98 * b:98 * b + 98, :],
                            in_=xs[:, b, 2 * S:2 * S + self.OWN].bitcast(F32))

        nc.compile()
        return nc


LAST_EXEC_NS = None


def run_chqs(input_img, k, d, weight, n_cores=8, runner=None, trace=False):
    B0, C, H, W = input_img.shape
    OWN = H // n_cores
    k2d = np.asarray(k, np.float32)[0, 0]
    d = np.asarray(d, np.float32)
    weight = np.asarray(weight, np.float32)
    offsG1 = _g_offsets(weight[0, 0])
    offsG2 = _g_offsets(weight[1, 0])
    bld = Builder(W, OWN, C, n_cores, offsG1, offsG2)
    nc = bld.build()
    NB, WPAD = bld.NB, bld.WPAD

    img = np.asarray(input_img, np.float32)[0]

    def to_planes(a):
        t = np.transpose(a, (0, 2, 1))
        if WPAD > W:
            t = np.concatenate(
                [t, np.repeat(t[:, W - 1:W, :], WPAD - W, axis=1)], axis=1)
        return np.ascontiguousarray(t)

    y_pl = to_planes(img)

    def band_of(pl, c):
        idx = np.clip(np.arange(OWN * c - 2 * S, OWN * c + OWN + 2 * S),
                      0, H - 1)
        return np.ascontiguousarray(pl[:, :, idx])

    y_bands = [band_of(y_pl, c) for c in range(n_cores)]
    mt = [np.full((98, 1), 1.0 if c == 0 else 0.0, np.float32)
          for c in range(n_cores)]
    mb = [np.full((98, 1), 1.0 if c == n_cores - 1 else 0.0, np.float32)
          for c in range(n_cores)]
    mti = [np.full((98, 2), 1 if c == 0 else 0, np.int32)
           for c in range(n_cores)]
    mbi = [np.full((98, 2), 1 if c == n_cores - 1 else 0, np.int32)
           for c in range(n_cores)]

    x_pl = y_pl.copy()
    for it in range(N_ITER):
        lamv = LAMBD / max(1e-4, float(BETA[it]))
        tabs = make_tables(k2d, d[it], weight, offsG1, offsG2)
        in_maps = []
        for c in range(n_cores):
            m = dict(tabs)
            m["x"] = band_of(x_pl, c)
            m["y"] = y_bands[c]
            m["lam"] = np.full((98, 1), lamv, np.float32)
            m["mtop"] = mt[c]
            m["mbot"] = mb[c]
            m["mti"] = mti[c]
            m["mbi"] = mbi[c]
            in_maps.append(m)
        if runner is None:
            res = run_bass_kernel_spmd(nc, in_maps, list(range(n_cores)),
                                       trace=trace)
            outs = res.results
            if res.exec_time_ns:
                global LAST_EXEC_NS
                LAST_EXEC_NS = (LAST_EXEC_NS or 0) + res.exec_time_ns
        else:
            outs = runner(nc, in_maps)
        for c in range(n_cores):
            x_pl[:, :, OWN * c:OWN * c + OWN] = outs[c]["o"]
    return np.ascontiguousarray(
        np.transpose(x_pl[:, :W, :], (0, 2, 1)))[None].astype(np.float32)


def kernel(input, k, d, weight):
    return run_chqs(input, k, d, weight, n_cores=8)
